# revision 1
# baseline (speedup 1.0000x reference)
"""Multi-head attention (B=2, L=2048, D=2048, 16 heads x 128) on 8 trn2 cores.

Sharding: tensor-parallel over heads (4 groups of 4 heads) x data-parallel
over batch (2) -> 8 cores.  Each core computes, for its (batch b, group g):
    hq = q_b @ Wq_g.T, hk = kv_b @ Wk_g.T, hv = kv_b @ Wv_g.T   (4 heads)
    per head: P = softmax(hq hk^T / sqrt(128)), o = P hv
    partial_out = concat_heads(o) @ Wo[:, g].T        [2048, 2048]
Host sums the 4 per-group partials for each batch.

All matmuls run as float32r (TF32-like, ~1.5e-4 relmax, full bf16-rate on
the PE).  The attention mask is all-ones per the problem spec and softmax
max-subtraction is skipped (logits are O(5), exp is safe in fp32).

Device layout notes (per core):
  qT/kvT   [2048 in, 2048 seq]   host-pretransposed, streamed in blocks
  hqT/hkT  [128 d, 4h x 2048 seq] in SBUF (d on partitions)
  hv       [128 k, 16 kt x 512(=4h x 128 d)] natural orientation
  scores^T [128 k-tile, 512 q] in PSUM -> exp on ACT -> SBUF
  AV:      o^T[128 d, 512 q] += hv_kt.T @ exp_kt  (PE psum accumulate)
  denom:   all-ones[128,128] stationary -> replicated [128, 512] sums
  Wo:      out[128 q, 512 dout] += o_chunk.T @ woT_chunk, per q-block
"""
import math
import sys

for _p in ("/opt/trn_rl_repo", "/root/.axon_site/_ro/trn_rl_repo"):
    if _p not in sys.path:
        sys.path.append(_p)

import numpy as np

B = 2
L = 2048           # LQ == LK
DIN = 2048
NH = 16            # total heads
HL = 4             # heads per core
D = 128            # head dim
HD = HL * D        # 512, head-group width
DOUT = 2048
NC_ = 8            # cores
NCH = DIN // 128   # 16 contraction chunks
NQ = 4             # q blocks of 512
QB = 512
NKT = L // 128     # 16 key tiles
DEN_PE_KT = 8     # key tiles whose denominator sum runs on the PE (rest on DVE)

_CACHE = {}


def _build_nc():
    import concourse.bacc as bacc
    import concourse.mybir as mybir
    import concourse.tile as tile

    F32R = mybir.dt.float32r
    F32 = mybir.dt.float32

    nc = bacc.Bacc("TRN2", target_bir_lowering=False, debug=False)
    qT = nc.dram_tensor("qT", [DIN, L], F32R, kind="ExternalInput").ap()
    kvT = nc.dram_tensor("kvT", [DIN, L], F32R, kind="ExternalInput").ap()
    wqT = nc.dram_tensor("wqT", [DIN, HD], F32R, kind="ExternalInput").ap()
    wkT = nc.dram_tensor("wkT", [DIN, HD], F32R, kind="ExternalInput").ap()
    wvT = nc.dram_tensor("wvT", [DIN, HD], F32R, kind="ExternalInput").ap()
    woT = nc.dram_tensor("woT", [HD, DOUT], F32R, kind="ExternalInput").ap()
    allones = nc.dram_tensor("allones", [128, 128], F32R, kind="ExternalInput").ap()
    out = nc.dram_tensor("out", [L, DOUT], F32R, kind="ExternalOutput").ap()

    EXP = mybir.ActivationFunctionType.Exp
    COPY = mybir.ActivationFunctionType.Copy

    with tile.TileContext(nc) as tc:
        with (
            nc.allow_low_precision(reason="float32r tiles are 4-byte fp32"),
            tc.tile_pool(name="persist", bufs=1) as pp,
            tc.tile_pool(name="psum", bufs=2, space="PSUM") as psp,
        ):
            hq_sb = pp.tile([128, HL * L], F32R, tag="hq")
            hk_sb = pp.tile([128, HL * L], F32R, tag="hk")
            hv_sb = pp.tile([128, NKT * HD], F32R, tag="hv")
            ones_sb = pp.tile([128, 128], F32R, tag="ones")
            nc.gpsimd.dma_start(out=ones_sb[:], in_=allones)

            # ---------------- projections ----------------
            with tc.tile_pool(name="proj", bufs=1) as jp:
                for pass_i, (w_dram, x_dram, dst) in enumerate(
                    [(wqT, qT, hq_sb), (wkT, kvT, hk_sb), (wvT, kvT, hv_sb)]
                ):
                    w_sb = jp.tile([128, NCH * HD], F32R, tag="w", bufs=2, name=f"w{pass_i}")
                    for c in range(NCH):
                        nc.gpsimd.dma_start(
                            out=w_sb[:, c * HD : (c + 1) * HD],
                            in_=w_dram[c * 128 : (c + 1) * 128, :],
                        )
                    is_v = pass_i == 2
                    for n in range(NQ):
                        # j0/j3 share one wide pp0 tile (bank-aligned halves) so
                        # every accumulator tag stays double-buffered across n.
                        acc03 = psp.tile([128, 2 * QB], F32, tag="pp0", name="acc03")
                        acc1 = psp.tile([128, QB], F32, tag="pp1", name="acc1")
                        acc2 = psp.tile([128, QB], F32, tag="pp2", name="acc2")
                        accs = [acc03[:, 0:QB], acc1[:], acc2[:], acc03[:, QB : 2 * QB]]
                        for cs in range(NCH // 4):
                            # 1 MiB super-block: 4 contraction chunks per DMA
                            sblk = jp.tile([128, 4 * QB], F32R, tag="blk", bufs=5, name="sblk")
                            nc.sync.dma_start(
                                out=sblk.rearrange("p (c q) -> p c q", q=QB),
                                in_=x_dram[
                                    cs * 512 : (cs + 1) * 512, n * QB : (n + 1) * QB
                                ].rearrange("(c p) q -> p c q", p=128),
                            )
                            for ci in range(4):
                                c = cs * 4 + ci
                                blk = sblk[:, ci * QB : (ci + 1) * QB]
                                for j in range(4):
                                    if is_v:
                                        # hv[k, d]: lhsT = kv block cols, rhs = w chunk
                                        nc.tensor.matmul(
                                            accs[j][:],
                                            blk[:, j * 128 : (j + 1) * 128],
                                            w_sb[:, c * HD : (c + 1) * HD],
                                            start=(c == 0),
                                            stop=(c == NCH - 1),
                                        )
                                    else:
                                        # hxT[d, q]: lhsT = w chunk head j, rhs = x block
                                        nc.tensor.matmul(
                                            accs[j][:],
                                            w_sb[:, c * HD + j * 128 : c * HD + (j + 1) * 128],
                                            blk[:],
                                            start=(c == 0),
                                            stop=(c == NCH - 1),
                                        )
                        for j in range(4):
                            if is_v:
                                # kt = n*4+j holds [128 k, 512(=4h x 128 d)]
                                nc.scalar.activation(
                                    dst[:, (n * 4 + j) * HD : (n * 4 + j + 1) * HD],
                                    accs[j][:],
                                    COPY,
                                )
                            else:
                                nc.scalar.activation(
                                    dst[:, j * L + n * QB : j * L + (n + 1) * QB],
                                    accs[j][:],
                                    COPY,
                                )

            # ---------------- attention + Wo ----------------
            with tc.tile_pool(name="attn", bufs=1) as ap:
                wo_sb = ap.tile([128, HL * DOUT], F32R, tag="wo", bufs=1, name="wo")
                for h in range(HL):
                    nc.gpsimd.dma_start(
                        out=wo_sb[:, h * DOUT : (h + 1) * DOUT],
                        in_=woT[h * 128 : (h + 1) * 128, :],
                    )
                def flush(st):
                    # deferred normalization of the previous (n, h) iteration:
                    # fold DVE partials into ps_d, reciprocal, scale AV output.
                    _, h_, ps_o_, ps_d_, d128_, o_sb_ = st
                    if d128_ is not None:
                        nc.tensor.matmul(
                            ps_d_[:], ones_sb[:], d128_[:], start=False, stop=True
                        )
                    recip = ap.tile([128, QB], F32, tag="recip", bufs=2, name="recip")
                    nc.vector.reciprocal_approx_fast(out=recip[:], in_=ps_d_[:])
                    nc.vector.tensor_mul(
                        out=o_sb_[:, h_ * QB : (h_ + 1) * QB],
                        in0=ps_o_[:],
                        in1=recip[:],
                    )

                def emit_wo_group(n_, o_sb_, g, on_act=False):
                    # one Wo output group (qtl, mp) for q block n_: 8 matmuls
                    qtl, mp = divmod(g, 2)
                    ps_f = psp.tile([128, 2 * QB], F32, tag="pp0", name="ps_f")
                    for h_ in range(HL):
                        for t in range(2):
                            m = 2 * mp + t
                            nc.tensor.matmul(
                                ps_f[:, t * QB : (t + 1) * QB],
                                o_sb_[:, h_ * QB + qtl * 128 : h_ * QB + (qtl + 1) * 128],
                                wo_sb[:, h_ * DOUT + m * QB : h_ * DOUT + (m + 1) * QB],
                                start=(h_ == 0),
                                stop=(h_ == HL - 1),
                            )
                    stage = ap.tile([128, 2 * QB], F32R, tag="stage", bufs=2, name="stage")
                    if on_act:
                        nc.scalar.activation(stage[:], ps_f[:], COPY)
                    else:
                        nc.vector.tensor_copy(out=stage[:], in_=ps_f[:])
                    nc.sync.dma_start(
                        out=out[
                            n_ * QB + qtl * 128 : n_ * QB + (qtl + 1) * 128,
                            mp * 2 * QB : (mp + 1) * 2 * QB,
                        ],
                        in_=stage[:],
                    )

                pending = None
                o_tiles = {}
                for n in range(NQ):
                    o_sb = ap.tile([128, HL * QB], F32R, tag="o", bufs=2, name="o")
                    o_tiles[n] = o_sb
                    for h in range(HL):
                        hq_sl = hq_sb[:, h * L + n * QB : h * L + (n + 1) * QB]
                        ps_o = psp.tile([128, QB], F32, tag="pp1", name="ps_o")
                        ps_d = psp.tile([128, QB], F32, tag="pp2", name="ps_d")
                        exp_half = [None, None]
                        # 9 cycles: scores/exp for pair p, AV lagged one pair
                        # behind (so the PE never races ACT's exp output), one
                        # denominator matmul per mid cycle.
                        for p in range(9):
                            if p < 8:
                                half = p // 4
                                if p % 4 == 0:
                                    exp_half[half] = ap.tile(
                                        [128, 8 * QB], F32R, tag="exp", bufs=3, name="exp"
                                    )
                                off = (p % 4) * 2 * QB
                                ps_s = psp.tile([128, 2 * QB], F32, tag="pp0", name="ps_s")
                                for t in range(2):
                                    kt = 2 * p + t
                                    nc.tensor.matmul(
                                        ps_s[:, t * QB : (t + 1) * QB],
                                        hk_sb[:, h * L + kt * 128 : h * L + (kt + 1) * 128],
                                        hq_sl,
                                        start=True,
                                        stop=True,
                                    )
                                nc.scalar.activation(
                                    exp_half[half][:, off : off + 2 * QB], ps_s[:], EXP
                                )
                            if p > 0:
                                for t in range(2):
                                    kt = 2 * (p - 1) + t
                                    e_sl = exp_half[kt // 8][
                                        :, (kt % 8) * QB : (kt % 8 + 1) * QB
                                    ]
                                    nc.tensor.matmul(
                                        ps_o[:],
                                        hv_sb[:, kt * HD + h * 128 : kt * HD + (h + 1) * 128],
                                        e_sl,
                                        start=(kt == 0),
                                        stop=(kt == NKT - 1),
                                    )
                                nc.tensor.matmul(
                                    ps_d[:],
                                    ones_sb[:],
                                    exp_half[0][:, (p - 1) * QB : p * QB],
                                    start=(p == 1),
                                    stop=False,
                                )
                            if p == 4:
                                # mid-iteration: normalize the previous (n, h)
                                # (its DVE partials had most of an iteration to
                                # finish), then slot in a Wo group of block n-1.
                                if pending is not None:
                                    flush(pending)
                                    pending = None
                                if n > 0:
                                    emit_wo_group(n - 1, o_tiles[n - 1], 2 * h)
                        # DVE elementwise sums of the second-half exp chunks;
                        # folded into ps_d by flush() half an iteration later.
                        d128 = ap.tile([128, QB], F32R, tag="d128", bufs=1, name="d128")
                        eh = exp_half[1]
                        nc.vector.tensor_add(
                            out=d128[:], in0=eh[:, 0:QB], in1=eh[:, QB : 2 * QB]
                        )
                        for i in range(2, 8):
                            nc.vector.tensor_add(
                                out=d128[:], in0=d128[:], in1=eh[:, i * QB : (i + 1) * QB]
                            )
                        if n > 0:
                            emit_wo_group(n - 1, o_tiles[n - 1], 2 * h + 1)
                        pending = (n, h, ps_o, ps_d, d128, o_sb)
                    if n > 0:
                        o_tiles.pop(n - 1)
                flush(pending)
                o_last = o_tiles.pop(NQ - 1)
                for g in range(8):
                    emit_wo_group(NQ - 1, o_last, g)
    nc.compile()
    return nc


def _get_nc():
    if "nc" not in _CACHE:
        _CACHE["nc"] = _build_nc()
    return _CACHE["nc"]


def make_in_maps(query, key_value, Wq, Wk, Wv, Wo):
    scale = 1.0 / math.sqrt(D)
    f32 = np.float32
    allones = np.ones((128, 128), f32)
    in_maps = []
    qT = [np.ascontiguousarray(query[b].T.astype(f32)) for b in range(B)]
    kvT = [np.ascontiguousarray(key_value[b].T.astype(f32)) for b in range(B)]
    for core in range(NC_):
        b, g = divmod(core, NC_ // B)
        sl = slice(g * HD, (g + 1) * HD)
        in_maps.append(
            {
                "qT": qT[b],
                "kvT": kvT[b],
                "wqT": np.ascontiguousarray((Wq[sl, :] * scale).T.astype(f32)),
                "wkT": np.ascontiguousarray(Wk[sl, :].T.astype(f32)),
                "wvT": np.ascontiguousarray(Wv[sl, :].T.astype(f32)),
                "woT": np.ascontiguousarray(Wo[:, sl].T.astype(f32)),
                "allones": allones,
            }
        )
    return in_maps


def _numpy_fallback(query, key_value, attention_mask, Wq, Wk, Wv, Wo):
    # Only reached if the mask is not all-ones (never per the problem spec).
    q64, kv64 = query.astype(np.float64), key_value.astype(np.float64)
    hq = (q64 @ Wq.T.astype(np.float64)).reshape(B, L, NH, D).transpose(0, 2, 1, 3)
    hk = (kv64 @ Wk.T.astype(np.float64)).reshape(B, L, NH, D).transpose(0, 2, 1, 3)
    hv = (kv64 @ Wv.T.astype(np.float64)).reshape(B, L, NH, D).transpose(0, 2, 1, 3)
    s = np.einsum("bhqd,bhkd->bhqk", hq, hk) / math.sqrt(D)
    mask = attention_mask[:, None, :, :]
    s = np.where(mask, s, -np.inf)
    s = s - s.max(axis=-1, keepdims=True)
    e = np.exp(s)
    p = e / np.maximum(e.sum(axis=-1, keepdims=True), 1e-300)
    p = np.where(mask, p, 0.0)
    o = np.einsum("bhqk,bhkd->bhqd", p, hv)
    o = o.transpose(0, 2, 1, 3).reshape(B, L, NH * D)
    return (o @ Wo.T.astype(np.float64)).astype(np.float32)


def kernel(query, key_value, attention_mask, Wq, Wk, Wv, Wo):
    query = np.asarray(query)
    key_value = np.asarray(key_value)
    attention_mask = np.asarray(attention_mask)
    Wq, Wk, Wv, Wo = (np.asarray(a) for a in (Wq, Wk, Wv, Wo))

    if not attention_mask.all():
        return _numpy_fallback(query, key_value, attention_mask, Wq, Wk, Wv, Wo)

    from concourse.bass_utils import run_bass_kernel_spmd

    nc = _get_nc()
    in_maps = make_in_maps(query, key_value, Wq, Wk, Wv, Wo)
    res = run_bass_kernel_spmd(nc, in_maps, list(range(NC_))).results
    out = np.zeros((B, L, DOUT), np.float32)
    for core in range(NC_):
        b = core // (NC_ // B)
        out[b] += res[core]["out"]
    return out



# revision 2
# speedup vs baseline: 1.3181x; 1.3181x over previous
"""Multi-head attention (B=2, L=2048, D=2048, 16 heads x 128) on 8 trn2 cores.

Sharding: tensor-parallel over heads (4 groups of 4 heads) x data-parallel
over batch (2) -> 8 cores.  Each core computes, for its (batch b, group g):
    hq = q_b @ Wq_g.T, hk = kv_b @ Wk_g.T, hv = kv_b @ Wv_g.T   (4 heads)
    per head: P = softmax(hq hk^T / sqrt(128)), o = P hv
    partial_out = concat_heads(o) @ Wo[:, g].T        [2048, 2048]
Host sums the 4 per-group partials for each batch.

All matmuls run in bf16 (fp32 PSUM accumulation).  bf16 stationary weight
loads are 1 cycle/row on the PE (vs ~4 for float32r), which removes most of
the LDWEIGHTS overhead that dominated the f32r version; bf16 also halves
DMA traffic.  End-to-end numeric error ~5e-3 (tolerance 2e-2).

The softmax denominator is computed off the PE: a bf16 pairwise tree of
DVE adds folds the 16 exp tiles into one [128, 512] partial, and a single
ones-matmul on the PE does the final cross-partition reduction.

Device layout (per core):
  xblk     [128, 16c x 512]   column block of qT/kvT, streamed per n
  hqT/hkT  [128 d, 4h x 2048 seq]  (d on partitions)
  hv       [128 k, 16 kt x 512(=4h x 128 d)]
  scores^T [128 k-tile, 2x512 q] in PSUM -> exp on ACT -> SBUF bf16
  AV:      o^T[128 d, 512 q] += hv_kt.T @ exp_kt  (PSUM accumulate)
  denom:   DVE bf16 tree -> d128; ones-matmul -> ps_d; recip+mul on DVE
  Wo:      out[128 q, 2048 dout] += o_chunk.T @ woT_chunk, per q-tile
"""
import math
import sys

for _p in ("/opt/trn_rl_repo", "/root/.axon_site/_ro/trn_rl_repo"):
    if _p not in sys.path:
        sys.path.append(_p)

import numpy as np
import ml_dtypes

B = 2
L = 2048           # LQ == LK
DIN = 2048
NH = 16            # total heads
HL = 4             # heads per core
D = 128            # head dim
HD = HL * D        # 512, head-group width
DOUT = 2048
NC_ = 8            # cores
NCH = DIN // 128   # 16 contraction chunks
NQ = 4             # q blocks of 512
QB = 512
NKT = L // 128     # 16 key tiles

_CACHE = {}


def _build_nc():
    import concourse.bacc as bacc
    import concourse.mybir as mybir
    import concourse.tile as tile

    BF16 = mybir.dt.bfloat16
    F32 = mybir.dt.float32

    nc = bacc.Bacc("TRN2", target_bir_lowering=False, debug=False)
    qT = nc.dram_tensor("qT", [DIN, L], BF16, kind="ExternalInput").ap()
    kvT = nc.dram_tensor("kvT", [DIN, L], BF16, kind="ExternalInput").ap()
    wqT = nc.dram_tensor("wqT", [DIN, HD], BF16, kind="ExternalInput").ap()
    wkT = nc.dram_tensor("wkT", [DIN, HD], BF16, kind="ExternalInput").ap()
    wvT = nc.dram_tensor("wvT", [DIN, HD], BF16, kind="ExternalInput").ap()
    woT = nc.dram_tensor("woT", [HD, DOUT], BF16, kind="ExternalInput").ap()
    allones = nc.dram_tensor("allones", [128, 128], BF16, kind="ExternalInput").ap()
    out = nc.dram_tensor("out", [L, DOUT], BF16, kind="ExternalOutput").ap()

    EXP = mybir.ActivationFunctionType.Exp
    COPY = mybir.ActivationFunctionType.Copy

    with tile.TileContext(nc) as tc:
        with (
            nc.allow_low_precision(reason="bf16 matmuls, ~5e-3 relmax vs 2e-2 tol"),
            tc.tile_pool(name="persist", bufs=1) as pp,
            tc.tile_pool(name="psum", bufs=2, space="PSUM") as psp,
        ):
            hq_sb = pp.tile([128, HL * L], BF16, tag="hq")
            hk_sb = pp.tile([128, HL * L], BF16, tag="hk")
            hv_sb = pp.tile([128, NKT * HD], BF16, tag="hv")
            o_sb = pp.tile([128, HL * L], BF16, tag="o")
            wo_sb = pp.tile([128, HL * DOUT], BF16, tag="wo")
            ones_sb = pp.tile([128, 128], BF16, tag="ones")
            nc.gpsimd.dma_start(out=ones_sb[:], in_=allones)

            # ---------------- projections ----------------
            with tc.tile_pool(name="proj", bufs=1) as jp:
                w_sbs = {}
                for nm, w_dram in (("wk", wkT), ("wv", wvT), ("wq", wqT)):
                    w_sb = jp.tile([128, NCH * HD], BF16, tag=f"w_{nm}", name=nm)
                    nc.gpsimd.dma_start(
                        out=w_sb.rearrange("p (c m) -> p c m", m=HD),
                        in_=w_dram.rearrange("(c p) m -> p c m", p=128),
                    )
                    w_sbs[nm] = w_sb
                for h in range(HL):
                    nc.gpsimd.dma_start(
                        out=wo_sb[:, h * DOUT : (h + 1) * DOUT],
                        in_=woT[h * 128 : (h + 1) * 128, :],
                    )

                def proj_block(x_sb, w_sb, accs, is_v):
                    for c in range(NCH):
                        for j in range(4):
                            if is_v:
                                nc.tensor.matmul(
                                    accs[j][:],
                                    x_sb[:, c * HD + j * 128 : c * HD + (j + 1) * 128],
                                    w_sb[:, c * HD : (c + 1) * HD],
                                    start=(c == 0),
                                    stop=(c == NCH - 1),
                                )
                            else:
                                nc.tensor.matmul(
                                    accs[j][:],
                                    w_sb[:, c * HD + j * 128 : c * HD + (j + 1) * 128],
                                    x_sb[:, c * HD : (c + 1) * HD],
                                    start=(c == 0),
                                    stop=(c == NCH - 1),
                                )

                def new_accs():
                    a = psp.tile([128, 2 * QB], F32, tag="ps", name="accA")
                    b = psp.tile([128, 2 * QB], F32, tag="ps2", name="accB")
                    return [a[:, 0:QB], a[:, QB : 2 * QB], b[:, 0:QB], b[:, QB : 2 * QB]]

                def load_xblk(x_dram, n):
                    xblk = jp.tile(
                        [128, NCH * QB], BF16, tag="xblk", bufs=2, name="xblk"
                    )
                    nc.sync.dma_start(
                        out=xblk.rearrange("p (c q) -> p c q", q=QB),
                        in_=x_dram[:, n * QB : (n + 1) * QB].rearrange(
                            "(c p) q -> p c q", p=128
                        ),
                    )
                    return xblk

                for n in range(NQ):
                    xblk = load_xblk(kvT, n)
                    # K projection: hk^T[d, seq], stationary = wk chunk
                    accs = new_accs()
                    proj_block(xblk, w_sbs["wk"], accs, is_v=False)
                    for j in range(4):
                        nc.scalar.activation(
                            hk_sb[:, j * L + n * QB : j * L + (n + 1) * QB],
                            accs[j][:],
                            COPY,
                        )
                    # V projection: hv[k, 4h*128d], stationary = kv chunk cols
                    accs = new_accs()
                    proj_block(xblk, w_sbs["wv"], accs, is_v=True)
                    for j in range(4):
                        nc.scalar.activation(
                            hv_sb[:, (n * 4 + j) * HD : (n * 4 + j + 1) * HD],
                            accs[j][:],
                            COPY,
                        )
                for n in range(NQ):
                    xblk = load_xblk(qT, n)
                    accs = new_accs()
                    proj_block(xblk, w_sbs["wq"], accs, is_v=False)
                    for j in range(4):
                        nc.scalar.activation(
                            hq_sb[:, j * L + n * QB : j * L + (n + 1) * QB],
                            accs[j][:],
                            COPY,
                        )

            # ---------------- attention ----------------
            with tc.tile_pool(name="attn", bufs=1) as ap:
                def flush(st):
                    # deferred normalization of the previous (n, h) iteration
                    h_, n_, ps_o_, ps_d_ = st
                    recip = ap.tile([128, QB], F32, tag="recip", bufs=2, name="recip")
                    nc.vector.reciprocal_approx_fast(out=recip[:], in_=ps_d_)
                    nc.vector.tensor_mul(
                        out=o_sb[:, h_ * L + n_ * QB : h_ * L + (n_ + 1) * QB],
                        in0=ps_o_,
                        in1=recip[:],
                    )

                pending = None
                for n in range(NQ):
                    for h in range(HL):
                        hq_sl = hq_sb[:, h * L + n * QB : h * L + (n + 1) * QB]
                        ps_opd = psp.tile([128, 2 * QB], F32, tag="ps2", name="ps_opd")
                        ps_o = ps_opd[:, 0:QB]
                        ps_d = ps_opd[:, QB : 2 * QB]
                        exp_half = [None, None]
                        # bf16 pairwise tree partials for the denominator
                        tl = {}  # level -> list of tiles
                        for p in range(9):
                            if p < 8:
                                half = p // 4
                                if p % 4 == 0:
                                    exp_half[half] = ap.tile(
                                        [128, 8 * QB], BF16, tag="exp", bufs=3,
                                        name="exp",
                                    )
                                off = (p % 4) * 2 * QB
                                ps_s = psp.tile(
                                    [128, 2 * QB], F32, tag="ps", name="ps_s"
                                )
                                for t in range(2):
                                    kt = 2 * p + t
                                    nc.tensor.matmul(
                                        ps_s[:, t * QB : (t + 1) * QB],
                                        hk_sb[:, h * L + kt * 128 : h * L + (kt + 1) * 128],
                                        hq_sl,
                                        start=True,
                                        stop=True,
                                    )
                                nc.scalar.activation(
                                    exp_half[half][:, off : off + 2 * QB], ps_s[:], EXP
                                )
                                # level-0 tree add over this exp pair
                                t0 = ap.tile([128, QB], BF16, tag="t0", bufs=2, name="t0")
                                nc.vector.tensor_add(
                                    out=t0[:],
                                    in0=exp_half[half][:, off : off + QB],
                                    in1=exp_half[half][:, off + QB : off + 2 * QB],
                                )
                                tl.setdefault(0, []).append(t0)
                                # fold higher levels as pairs complete
                                lv = 0
                                while len(tl.get(lv, [])) == 2:
                                    a, b = tl.pop(lv)
                                    nxt = ap.tile(
                                        [128, QB], BF16, tag=f"t{lv+1}", bufs=2,
                                        name=f"t{lv+1}",
                                    )
                                    nc.vector.tensor_add(out=nxt[:], in0=a[:], in1=b[:])
                                    tl.setdefault(lv + 1, []).append(nxt)
                                    lv += 1
                            if p > 0:
                                for t in range(2):
                                    kt = 2 * (p - 1) + t
                                    e_sl = exp_half[kt // 8][
                                        :, (kt % 8) * QB : (kt % 8 + 1) * QB
                                    ]
                                    nc.tensor.matmul(
                                        ps_o,
                                        hv_sb[:, kt * HD + h * 128 : kt * HD + (h + 1) * 128],
                                        e_sl,
                                        start=(kt == 0),
                                        stop=(kt == NKT - 1),
                                    )
                            if p == 4 and pending is not None:
                                flush(pending)
                                pending = None
                        d128 = tl[3][0]
                        # cross-partition reduction of the tree partial
                        nc.tensor.matmul(
                            ps_d, ones_sb[:], d128[:], start=True, stop=True
                        )
                        pending = (h, n, ps_o, ps_d)
                flush(pending)

            # ---------------- Wo ----------------
            with tc.tile_pool(name="wop", bufs=1) as wp:
                for qt in range(16):
                    accA = psp.tile([128, 2 * QB], F32, tag="ps", name="woA")
                    accB = psp.tile([128, 2 * QB], F32, tag="ps2", name="woB")
                    halves = [
                        accA[:, 0:QB],
                        accA[:, QB : 2 * QB],
                        accB[:, 0:QB],
                        accB[:, QB : 2 * QB],
                    ]
                    for h in range(HL):
                        lhsT = o_sb[:, h * L + qt * 128 : h * L + (qt + 1) * 128]
                        for m in range(4):
                            nc.tensor.matmul(
                                halves[m],
                                lhsT,
                                wo_sb[:, h * DOUT + m * QB : h * DOUT + (m + 1) * QB],
                                start=(h == 0),
                                stop=(h == HL - 1),
                            )
                    stage = wp.tile(
                        [128, 4 * QB], BF16, tag="stage", bufs=2, name="stage"
                    )
                    nc.scalar.activation(stage[:, 0 : 2 * QB], accA[:], COPY)
                    nc.scalar.activation(stage[:, 2 * QB : 4 * QB], accB[:], COPY)
                    nc.sync.dma_start(
                        out=out[qt * 128 : (qt + 1) * 128, :],
                        in_=stage[:],
                    )
    nc.compile()
    return nc


def _get_nc():
    if "nc" not in _CACHE:
        _CACHE["nc"] = _build_nc()
    return _CACHE["nc"]


def make_in_maps(query, key_value, Wq, Wk, Wv, Wo):
    scale = 1.0 / math.sqrt(D)
    bf = ml_dtypes.bfloat16
    allones = np.ones((128, 128), bf)
    in_maps = []
    qT = [np.ascontiguousarray(query[b].T.astype(bf)) for b in range(B)]
    kvT = [np.ascontiguousarray(key_value[b].T.astype(bf)) for b in range(B)]
    for core in range(NC_):
        b, g = divmod(core, NC_ // B)
        sl = slice(g * HD, (g + 1) * HD)
        in_maps.append(
            {
                "qT": qT[b],
                "kvT": kvT[b],
                "wqT": np.ascontiguousarray((Wq[sl, :] * scale).T.astype(bf)),
                "wkT": np.ascontiguousarray(Wk[sl, :].T.astype(bf)),
                "wvT": np.ascontiguousarray(Wv[sl, :].T.astype(bf)),
                "woT": np.ascontiguousarray(Wo[:, sl].T.astype(bf)),
                "allones": allones,
            }
        )
    return in_maps


def _numpy_fallback(query, key_value, attention_mask, Wq, Wk, Wv, Wo):
    # Only reached if the mask is not all-ones (never per the problem spec).
    q64, kv64 = query.astype(np.float64), key_value.astype(np.float64)
    hq = (q64 @ Wq.T.astype(np.float64)).reshape(B, L, NH, D).transpose(0, 2, 1, 3)
    hk = (kv64 @ Wk.T.astype(np.float64)).reshape(B, L, NH, D).transpose(0, 2, 1, 3)
    hv = (kv64 @ Wv.T.astype(np.float64)).reshape(B, L, NH, D).transpose(0, 2, 1, 3)
    s = np.einsum("bhqd,bhkd->bhqk", hq, hk) / math.sqrt(D)
    mask = attention_mask[:, None, :, :]
    s = np.where(mask, s, -np.inf)
    s = s - s.max(axis=-1, keepdims=True)
    e = np.exp(s)
    p = e / np.maximum(e.sum(axis=-1, keepdims=True), 1e-300)
    p = np.where(mask, p, 0.0)
    o = np.einsum("bhqk,bhkd->bhqd", p, hv)
    o = o.transpose(0, 2, 1, 3).reshape(B, L, NH * D)
    return (o @ Wo.T.astype(np.float64)).astype(np.float32)


def kernel(query, key_value, attention_mask, Wq, Wk, Wv, Wo):
    query = np.asarray(query)
    key_value = np.asarray(key_value)
    attention_mask = np.asarray(attention_mask)
    Wq, Wk, Wv, Wo = (np.asarray(a) for a in (Wq, Wk, Wv, Wo))

    if not attention_mask.all():
        return _numpy_fallback(query, key_value, attention_mask, Wq, Wk, Wv, Wo)

    from concourse.bass_utils import run_bass_kernel_spmd

    nc = _get_nc()
    in_maps = make_in_maps(query, key_value, Wq, Wk, Wv, Wo)
    res = run_bass_kernel_spmd(nc, in_maps, list(range(NC_))).results
    out = np.zeros((B, L, DOUT), np.float32)
    for core in range(NC_):
        b = core // (NC_ // B)
        out[b] += res[core]["out"].astype(np.float32)
    return out


# revision 7
# speedup vs baseline: 1.3359x; 1.0135x over previous
"""Multi-head attention (B=2, L=2048, D=2048, 16 heads x 128) on 8 trn2 cores.

Sharding: tensor-parallel over heads (4 groups of 4 heads) x data-parallel
over batch (2) -> 8 cores.  Each core computes, for its (batch b, group g):
    hq = q_b @ Wq_g.T, hk = kv_b @ Wk_g.T, hv = kv_b @ Wv_g.T   (4 heads)
    per head: P = softmax(hq hk^T / sqrt(128)), o = P hv
    partial_out = concat_heads(o) @ Wo[:, g].T        [2048, 2048]
Host sums the 4 per-group partials for each batch.

All matmuls run in bf16 (fp32 PSUM accumulation).  bf16 stationary weight
loads are 1 cycle/row on the PE (vs ~4 for float32r), which removes most of
the LDWEIGHTS overhead that dominated the f32r version; bf16 also halves
DMA traffic.  End-to-end numeric error ~5e-3 (tolerance 2e-2).

The softmax denominator is computed off the PE: a bf16 pairwise tree of
DVE adds folds the 16 exp tiles into one [128, 512] partial, and a single
ones-matmul on the PE does the final cross-partition reduction.

Device layout (per core):
  xblk     [128, 16c x 512]   column block of qT/kvT, streamed per n
  hqT/hkT  [128 d, 4h x 2048 seq]  (d on partitions)
  hv       [128 k, 16 kt x 512(=4h x 128 d)]
  scores^T [128 k-tile, 2x512 q] in PSUM -> exp on ACT -> SBUF bf16
  AV:      o^T[128 d, 512 q] += hv_kt.T @ exp_kt  (PSUM accumulate)
  denom:   DVE bf16 tree -> d128; ones-matmul -> ps_d; recip+mul on DVE
  Wo:      out[128 q, 2048 dout] += o_chunk.T @ woT_chunk, per q-tile
"""
import math
import sys

for _p in ("/opt/trn_rl_repo", "/root/.axon_site/_ro/trn_rl_repo"):
    if _p not in sys.path:
        sys.path.append(_p)

import numpy as np
import ml_dtypes

B = 2
L = 2048           # LQ == LK
DIN = 2048
NH = 16            # total heads
HL = 4             # heads per core
D = 128            # head dim
HD = HL * D        # 512, head-group width
DOUT = 2048
NC_ = 8            # cores
NCH = DIN // 128   # 16 contraction chunks
NQ = 4             # q blocks of 512
QB = 512
NKT = L // 128     # 16 key tiles

_CACHE = {}


def _build_nc():
    import concourse.bacc as bacc
    import concourse.mybir as mybir
    import concourse.tile as tile

    BF16 = mybir.dt.bfloat16
    F32 = mybir.dt.float32

    nc = bacc.Bacc("TRN2", target_bir_lowering=False, debug=False)
    qT = nc.dram_tensor("qT", [DIN, L], BF16, kind="ExternalInput").ap()
    kvT = nc.dram_tensor("kvT", [DIN, L], BF16, kind="ExternalInput").ap()
    wqT = nc.dram_tensor("wqT", [DIN, HD], BF16, kind="ExternalInput").ap()
    wkT = nc.dram_tensor("wkT", [DIN, HD], BF16, kind="ExternalInput").ap()
    wvT = nc.dram_tensor("wvT", [DIN, HD], BF16, kind="ExternalInput").ap()
    woT = nc.dram_tensor("woT", [HD, DOUT], BF16, kind="ExternalInput").ap()
    allones = nc.dram_tensor("allones", [128, 128], BF16, kind="ExternalInput").ap()
    out = nc.dram_tensor("out", [L, DOUT], BF16, kind="ExternalOutput").ap()

    EXP = mybir.ActivationFunctionType.Exp
    COPY = mybir.ActivationFunctionType.Copy

    with tile.TileContext(nc) as tc:
        with (
            nc.allow_low_precision(reason="bf16 matmuls, ~5e-3 relmax vs 2e-2 tol"),
            tc.tile_pool(name="persist", bufs=1) as pp,
            tc.tile_pool(name="psum", bufs=2, space="PSUM") as psp,
        ):
            hq_sb = pp.tile([128, HL * L], BF16, tag="hq")
            hk_sb = pp.tile([128, HL * L], BF16, tag="hk")
            hv_sb = pp.tile([128, NKT * HD], BF16, tag="hv")
            o_sb = pp.tile([128, HL * L], BF16, tag="o")
            wo_sb = pp.tile([128, HL * DOUT], BF16, tag="wo")
            ones_sb = pp.tile([128, 128], BF16, tag="ones")
            nc.gpsimd.dma_start(out=ones_sb[:], in_=allones)

            # ---------------- projections ----------------
            with tc.tile_pool(name="proj", bufs=1) as jp:
                # wk and the first kv block gate the first matmul: stream both
                # in interleaved 4-chunk pieces so c=0 can start early.
                def load_w_piece(w_sb, w_dram, piece):
                    c0, c1 = piece * 4, (piece + 1) * 4
                    nc.gpsimd.dma_start(
                        out=w_sb[:, c0 * HD : c1 * HD].rearrange(
                            "p (c m) -> p c m", m=HD
                        ),
                        in_=w_dram[c0 * 128 : c1 * 128, :].rearrange(
                            "(c p) m -> p c m", p=128
                        ),
                    )

                w_sbs = {
                    nm: jp.tile([128, NCH * HD], BF16, tag=f"w_{nm}", name=nm)
                    for nm in ("wk", "wv", "wq")
                }
                xblk0 = jp.tile([128, NCH * QB], BF16, tag="xblk", bufs=2, name="xblk")

                def load_x_piece(xblk, x_dram, n, piece):
                    c0, c1 = piece * 4, (piece + 1) * 4
                    nc.sync.dma_start(
                        out=xblk[:, c0 * QB : c1 * QB].rearrange(
                            "p (c q) -> p c q", q=QB
                        ),
                        in_=x_dram[c0 * 128 : c1 * 128, n * QB : (n + 1) * QB].rearrange(
                            "(c p) q -> p c q", p=128
                        ),
                    )

                for piece in range(4):
                    load_w_piece(w_sbs["wk"], wkT, piece)
                    load_x_piece(xblk0, kvT, 0, piece)
                for piece in range(4):
                    load_w_piece(w_sbs["wv"], wvT, piece)
                for piece in range(4):
                    load_w_piece(w_sbs["wq"], wqT, piece)
                for h in range(HL):
                    nc.gpsimd.dma_start(
                        out=wo_sb[:, h * DOUT : (h + 1) * DOUT],
                        in_=woT[h * 128 : (h + 1) * 128, :],
                    )

                def proj_block(x_sb, w_sb, accs, is_v):
                    for c in range(NCH):
                        for j in range(4):
                            if is_v:
                                nc.tensor.matmul(
                                    accs[j][:],
                                    x_sb[:, c * HD + j * 128 : c * HD + (j + 1) * 128],
                                    w_sb[:, c * HD : (c + 1) * HD],
                                    start=(c == 0),
                                    stop=(c == NCH - 1),
                                )
                            else:
                                nc.tensor.matmul(
                                    accs[j][:],
                                    w_sb[:, c * HD + j * 128 : c * HD + (j + 1) * 128],
                                    x_sb[:, c * HD : (c + 1) * HD],
                                    start=(c == 0),
                                    stop=(c == NCH - 1),
                                )

                def new_accs():
                    a = psp.tile([128, 2 * QB], F32, tag="ps", name="accA")
                    b = psp.tile([128, 2 * QB], F32, tag="ps2", name="accB")
                    return [a[:, 0:QB], a[:, QB : 2 * QB], b[:, 0:QB], b[:, QB : 2 * QB]]

                def load_xblk(x_dram, n):
                    xblk = jp.tile(
                        [128, NCH * QB], BF16, tag="xblk", bufs=2, name="xblk"
                    )
                    nc.sync.dma_start(
                        out=xblk.rearrange("p (c q) -> p c q", q=QB),
                        in_=x_dram[:, n * QB : (n + 1) * QB].rearrange(
                            "(c p) q -> p c q", p=128
                        ),
                    )
                    return xblk

                for n in range(NQ):
                    xblk = xblk0 if n == 0 else load_xblk(kvT, n)
                    # K projection: hk^T[d, seq], stationary = wk chunk
                    accs = new_accs()
                    proj_block(xblk, w_sbs["wk"], accs, is_v=False)
                    for j in range(4):
                        nc.scalar.activation(
                            hk_sb[:, j * L + n * QB : j * L + (n + 1) * QB],
                            accs[j][:],
                            COPY,
                        )
                    # V projection: hv[k, 4h*128d], stationary = kv chunk cols
                    accs = new_accs()
                    proj_block(xblk, w_sbs["wv"], accs, is_v=True)
                    for j in range(4):
                        nc.scalar.activation(
                            hv_sb[:, (n * 4 + j) * HD : (n * 4 + j + 1) * HD],
                            accs[j][:],
                            COPY,
                        )
                for n in range(NQ):
                    xblk = load_xblk(qT, n)
                    accs = new_accs()
                    proj_block(xblk, w_sbs["wq"], accs, is_v=False)
                    for j in range(4):
                        # last pass drains on DVE so ACT is free for the
                        # first attention exps (and psum slots free sooner)
                        if n == NQ - 1:
                            nc.vector.tensor_copy(
                                out=hq_sb[:, j * L + n * QB : j * L + (n + 1) * QB],
                                in_=accs[j][:],
                            )
                        else:
                            nc.scalar.activation(
                                hq_sb[:, j * L + n * QB : j * L + (n + 1) * QB],
                                accs[j][:],
                                COPY,
                            )

            # ---------------- attention ----------------
            with tc.tile_pool(name="attn", bufs=1) as ap:
                def flush(st):
                    # deferred normalization of the previous (n, h) iteration
                    h_, n_, ps_o_, ps_d_ = st
                    recip = ap.tile([128, QB], F32, tag="recip", bufs=2, name="recip")
                    nc.vector.reciprocal_approx_fast(out=recip[:], in_=ps_d_)
                    nc.vector.tensor_mul(
                        out=o_sb[:, h_ * L + n_ * QB : h_ * L + (n_ + 1) * QB],
                        in0=ps_o_,
                        in1=recip[:],
                    )

                # Flat software pipeline over the 16 (n, h) units: AV lags
                # scores by one kt-pair within a unit; the last AV pair, the
                # ones-matmul (partition reduction) and the normalization of
                # unit u are deferred into unit u+1 so the PE never waits on
                # ACT/DVE at a unit boundary.
                units = [(n, h) for n in range(NQ) for h in range(HL)]
                state = {}  # u -> dict with ps_o, ps_d, exp_half, d128, (n, h)
                prev_flush = None

                def emit_av(u, pair):
                    st = state[u]
                    n_, h_ = st["nh"]
                    for t in range(2):
                        kt = 2 * pair + t
                        e_sl = st["exp_half"][kt // 8][
                            :, (kt % 8) * QB : (kt % 8 + 1) * QB
                        ]
                        nc.tensor.matmul(
                            st["ps_o"],
                            hv_sb[:, kt * HD + h_ * 128 : kt * HD + (h_ + 1) * 128],
                            e_sl,
                            start=(kt == 0),
                            stop=(kt == NKT - 1),
                        )

                for u, (n, h) in enumerate(units):
                    hq_sl = hq_sb[:, h * L + n * QB : h * L + (n + 1) * QB]
                    ps_opd = psp.tile([128, 2 * QB], F32, tag="ps2", name="ps_opd")
                    st = {
                        "nh": (n, h),
                        "ps_o": ps_opd[:, 0:QB],
                        "ps_d": ps_opd[:, QB : 2 * QB],
                        "exp_half": [None, None],
                    }
                    state[u] = st
                    tl = {}  # denominator tree: level -> pending tiles
                    for p in range(8):
                        half = p // 4
                        if p % 4 == 0:
                            st["exp_half"][half] = ap.tile(
                                [128, 8 * QB], BF16, tag="exp", bufs=3, name="exp"
                            )
                        off = (p % 4) * 2 * QB
                        ps_s = psp.tile([128, 2 * QB], F32, tag="ps", name="ps_s")
                        for t in range(2):
                            kt = 2 * p + t
                            nc.tensor.matmul(
                                ps_s[:, t * QB : (t + 1) * QB],
                                hk_sb[:, h * L + kt * 128 : h * L + (kt + 1) * 128],
                                hq_sl,
                                start=True,
                                stop=True,
                            )
                        nc.scalar.activation(
                            st["exp_half"][half][:, off : off + 2 * QB], ps_s[:], EXP
                        )
                        # level-0 tree add over this exp pair, fold-up when ready
                        t0 = ap.tile([128, QB], BF16, tag="t0", bufs=2, name="t0")
                        nc.vector.tensor_add(
                            out=t0[:],
                            in0=st["exp_half"][half][:, off : off + QB],
                            in1=st["exp_half"][half][:, off + QB : off + 2 * QB],
                        )
                        tl.setdefault(0, []).append(t0)
                        lv = 0
                        while len(tl.get(lv, [])) == 2:
                            a, b = tl.pop(lv)
                            nxt = ap.tile(
                                [128, QB], BF16, tag=f"t{lv+1}", bufs=2,
                                name=f"t{lv+1}",
                            )
                            nc.vector.tensor_add(out=nxt[:], in0=a[:], in1=b[:])
                            tl.setdefault(lv + 1, []).append(nxt)
                            lv += 1
                        # deferred work from the previous unit / this unit
                        if p == 0 and u > 0:
                            emit_av(u - 1, 7)
                        if p >= 1:
                            emit_av(u, p - 1)
                        if p == 1 and u > 0:
                            pst = state[u - 1]
                            nc.tensor.matmul(
                                pst["ps_d"], ones_sb[:], pst["d128"][:],
                                start=True, stop=True,
                            )
                            n_, h_ = pst["nh"]
                            prev_flush = (h_, n_, pst["ps_o"], pst["ps_d"])
                        if p == 4 and prev_flush is not None:
                            flush(prev_flush)
                            prev_flush = None
                    st["d128"] = tl[3][0]
                    if u > 0:
                        state.pop(u - 1)
                # drain the pipeline: last unit's AV tail, reduction, flushes
                last = len(units) - 1
                emit_av(last, 7)
                st = state[last]
                nc.tensor.matmul(
                    st["ps_d"], ones_sb[:], st["d128"][:], start=True, stop=True
                )
                if prev_flush is not None:  # unit last-1, ones'd at p1 of last
                    flush(prev_flush)
                n_, h_ = st["nh"]
                flush((h_, n_, st["ps_o"], st["ps_d"]))

            # ---------------- Wo ----------------
            with tc.tile_pool(name="wop", bufs=1) as wp:
                for qt in range(16):
                    accA = psp.tile([128, 2 * QB], F32, tag="ps", name="woA")
                    accB = psp.tile([128, 2 * QB], F32, tag="ps2", name="woB")
                    halves = [
                        accA[:, 0:QB],
                        accA[:, QB : 2 * QB],
                        accB[:, 0:QB],
                        accB[:, QB : 2 * QB],
                    ]
                    for h in range(HL):
                        lhsT = o_sb[:, h * L + qt * 128 : h * L + (qt + 1) * 128]
                        for m in range(4):
                            nc.tensor.matmul(
                                halves[m],
                                lhsT,
                                wo_sb[:, h * DOUT + m * QB : h * DOUT + (m + 1) * QB],
                                start=(h == 0),
                                stop=(h == HL - 1),
                            )
                    stage = wp.tile(
                        [128, 4 * QB], BF16, tag="stage", bufs=2, name="stage"
                    )
                    nc.scalar.activation(stage[:, 0 : 2 * QB], accA[:], COPY)
                    nc.scalar.activation(stage[:, 2 * QB : 4 * QB], accB[:], COPY)
                    nc.sync.dma_start(
                        out=out[qt * 128 : (qt + 1) * 128, :],
                        in_=stage[:],
                    )
    nc.compile()
    return nc


def _get_nc():
    if "nc" not in _CACHE:
        _CACHE["nc"] = _build_nc()
    return _CACHE["nc"]


def make_in_maps(query, key_value, Wq, Wk, Wv, Wo):
    scale = 1.0 / math.sqrt(D)
    bf = ml_dtypes.bfloat16
    allones = np.ones((128, 128), bf)
    in_maps = []
    qT = [np.ascontiguousarray(query[b].T.astype(bf)) for b in range(B)]
    kvT = [np.ascontiguousarray(key_value[b].T.astype(bf)) for b in range(B)]
    for core in range(NC_):
        b, g = divmod(core, NC_ // B)
        sl = slice(g * HD, (g + 1) * HD)
        in_maps.append(
            {
                "qT": qT[b],
                "kvT": kvT[b],
                "wqT": np.ascontiguousarray((Wq[sl, :] * scale).T.astype(bf)),
                "wkT": np.ascontiguousarray(Wk[sl, :].T.astype(bf)),
                "wvT": np.ascontiguousarray(Wv[sl, :].T.astype(bf)),
                "woT": np.ascontiguousarray(Wo[:, sl].T.astype(bf)),
                "allones": allones,
            }
        )
    return in_maps


def _numpy_fallback(query, key_value, attention_mask, Wq, Wk, Wv, Wo):
    # Only reached if the mask is not all-ones (never per the problem spec).
    q64, kv64 = query.astype(np.float64), key_value.astype(np.float64)
    hq = (q64 @ Wq.T.astype(np.float64)).reshape(B, L, NH, D).transpose(0, 2, 1, 3)
    hk = (kv64 @ Wk.T.astype(np.float64)).reshape(B, L, NH, D).transpose(0, 2, 1, 3)
    hv = (kv64 @ Wv.T.astype(np.float64)).reshape(B, L, NH, D).transpose(0, 2, 1, 3)
    s = np.einsum("bhqd,bhkd->bhqk", hq, hk) / math.sqrt(D)
    mask = attention_mask[:, None, :, :]
    s = np.where(mask, s, -np.inf)
    s = s - s.max(axis=-1, keepdims=True)
    e = np.exp(s)
    p = e / np.maximum(e.sum(axis=-1, keepdims=True), 1e-300)
    p = np.where(mask, p, 0.0)
    o = np.einsum("bhqk,bhkd->bhqd", p, hv)
    o = o.transpose(0, 2, 1, 3).reshape(B, L, NH * D)
    return (o @ Wo.T.astype(np.float64)).astype(np.float32)


def kernel(query, key_value, attention_mask, Wq, Wk, Wv, Wo):
    query = np.asarray(query)
    key_value = np.asarray(key_value)
    attention_mask = np.asarray(attention_mask)
    Wq, Wk, Wv, Wo = (np.asarray(a) for a in (Wq, Wk, Wv, Wo))

    if not attention_mask.all():
        return _numpy_fallback(query, key_value, attention_mask, Wq, Wk, Wv, Wo)

    from concourse.bass_utils import run_bass_kernel_spmd

    nc = _get_nc()
    in_maps = make_in_maps(query, key_value, Wq, Wk, Wv, Wo)
    res = run_bass_kernel_spmd(nc, in_maps, list(range(NC_))).results
    out = np.zeros((B, L, DOUT), np.float32)
    for core in range(NC_):
        b = core // (NC_ // B)
        out[b] += res[core]["out"].astype(np.float32)
    return out


# revision 14
# speedup vs baseline: 1.3657x; 1.0223x over previous
"""Multi-head attention (B=2, L=2048, D=2048, 16 heads x 128) on 8 trn2 cores.

Sharding: tensor-parallel over heads (4 groups of 4 heads) x data-parallel
over batch (2) -> 8 cores.  Each core computes, for its (batch b, group g):
    hq = q_b @ Wq_g.T, hk = kv_b @ Wk_g.T, hv = kv_b @ Wv_g.T   (4 heads)
    per head: P = softmax(hq hk^T / sqrt(128)), o = P hv
    partial_out = concat_heads(o) @ Wo[:, g].T        [2048, 2048]
Host sums the 4 per-group partials for each batch.

All matmuls run in bf16 (fp32 PSUM accumulation).  bf16 stationary weight
loads are 1 cycle/row on the PE (vs ~4 for float32r), which removes most of
the LDWEIGHTS overhead that dominated the f32r version; bf16 also halves
DMA traffic.  End-to-end numeric error ~5e-3 (tolerance 2e-2).

The softmax denominator is computed off the PE: a bf16 pairwise tree of
DVE adds folds the 16 exp tiles into one [128, 512] partial, and a single
ones-matmul on the PE does the final cross-partition reduction.

Device layout (per core):
  xblk     [128, 16c x 512]   column block of qT/kvT, streamed per n
  hqT/hkT  [128 d, 4h x 2048 seq]  (d on partitions)
  hv       [128 k, 16 kt x 512(=4h x 128 d)]
  scores^T [128 k-tile, 2x512 q] in PSUM -> exp on ACT -> SBUF bf16
  AV:      o^T[128 d, 512 q] += hv_kt.T @ exp_kt  (PSUM accumulate)
  denom:   DVE bf16 tree -> d128; ones-matmul -> ps_d; recip+mul on DVE
  Wo:      out[128 q, 2048 dout] += o_chunk.T @ woT_chunk, per q-tile
"""
import math
import sys

for _p in ("/opt/trn_rl_repo", "/root/.axon_site/_ro/trn_rl_repo"):
    if _p not in sys.path:
        sys.path.append(_p)

import numpy as np
import ml_dtypes

B = 2
L = 2048           # LQ == LK
DIN = 2048
NH = 16            # total heads
HL = 4             # heads per core
D = 128            # head dim
HD = HL * D        # 512, head-group width
DOUT = 2048
NC_ = 8            # cores
NCH = DIN // 128   # 16 contraction chunks
NQ = 4             # q blocks of 512
QB = 512
NKT = L // 128     # 16 key tiles

_CACHE = {}


def _build_nc():
    import concourse.bacc as bacc
    import concourse.mybir as mybir
    import concourse.tile as tile

    BF16 = mybir.dt.bfloat16
    F32 = mybir.dt.float32

    nc = bacc.Bacc("TRN2", target_bir_lowering=False, debug=False)
    qT = nc.dram_tensor("qT", [DIN, L], BF16, kind="ExternalInput").ap()
    kvT = nc.dram_tensor("kvT", [DIN, L], BF16, kind="ExternalInput").ap()
    wqT = nc.dram_tensor("wqT", [DIN, HD], BF16, kind="ExternalInput").ap()
    wkT = nc.dram_tensor("wkT", [DIN, HD], BF16, kind="ExternalInput").ap()
    wvT = nc.dram_tensor("wvT", [DIN, HD], BF16, kind="ExternalInput").ap()
    woT = nc.dram_tensor("woT", [HD, DOUT], BF16, kind="ExternalInput").ap()
    allones = nc.dram_tensor("allones", [128, 128], BF16, kind="ExternalInput").ap()
    out = nc.dram_tensor("out", [L, DOUT], BF16, kind="ExternalOutput").ap()

    EXP = mybir.ActivationFunctionType.Exp
    COPY = mybir.ActivationFunctionType.Copy

    with tile.TileContext(nc) as tc:
        with (
            nc.allow_low_precision(reason="bf16 matmuls, ~5e-3 relmax vs 2e-2 tol"),
            tc.tile_pool(name="persist", bufs=1) as pp,
            tc.tile_pool(name="psum", bufs=2, space="PSUM") as psp,
        ):
            hq_sb = pp.tile([128, HL * L], BF16, tag="hq")
            hk_sb = pp.tile([128, HL * L], BF16, tag="hk")
            hv_sb = pp.tile([128, NKT * HD], BF16, tag="hv")
            o_sb = pp.tile([128, HL * L], BF16, tag="o")
            wo_sb = pp.tile([128, HL * DOUT], BF16, tag="wo")
            ones_sb = pp.tile([128, 128], BF16, tag="ones")
            nc.gpsimd.dma_start(out=ones_sb[:], in_=allones)

            # ---------------- projections ----------------
            with tc.tile_pool(name="proj", bufs=1) as jp:
                # wk and the first kv block gate the first matmul: stream both
                # in interleaved 4-chunk pieces so c=0 can start early.
                def load_w_piece(w_sb, w_dram, piece):
                    c0, c1 = piece * 4, (piece + 1) * 4
                    nc.gpsimd.dma_start(
                        out=w_sb[:, c0 * HD : c1 * HD].rearrange(
                            "p (c m) -> p c m", m=HD
                        ),
                        in_=w_dram[c0 * 128 : c1 * 128, :].rearrange(
                            "(c p) m -> p c m", p=128
                        ),
                    )

                w_sbs = {
                    nm: jp.tile([128, NCH * HD], BF16, tag=f"w_{nm}", name=nm)
                    for nm in ("wk", "wv", "wq")
                }
                xblk0 = jp.tile([128, NCH * QB], BF16, tag="xblk", bufs=2, name="xblk")

                def load_x_piece(xblk, x_dram, n, piece):
                    c0, c1 = piece * 4, (piece + 1) * 4
                    nc.sync.dma_start(
                        out=xblk[:, c0 * QB : c1 * QB].rearrange(
                            "p (c q) -> p c q", q=QB
                        ),
                        in_=x_dram[c0 * 128 : c1 * 128, n * QB : (n + 1) * QB].rearrange(
                            "(c p) q -> p c q", p=128
                        ),
                    )

                for piece in range(4):
                    load_w_piece(w_sbs["wk"], wkT, piece)
                    load_x_piece(xblk0, kvT, 0, piece)
                for piece in range(4):
                    load_w_piece(w_sbs["wv"], wvT, piece)
                for piece in range(4):
                    load_w_piece(w_sbs["wq"], wqT, piece)
                for h in range(HL):
                    nc.gpsimd.dma_start(
                        out=wo_sb[:, h * DOUT : (h + 1) * DOUT],
                        in_=woT[h * 128 : (h + 1) * 128, :],
                    )

                def proj_block(x_sb, w_sb, accs, is_v):
                    for c in range(NCH):
                        for j in range(4):
                            if is_v:
                                nc.tensor.matmul(
                                    accs[j][:],
                                    x_sb[:, c * HD + j * 128 : c * HD + (j + 1) * 128],
                                    w_sb[:, c * HD : (c + 1) * HD],
                                    start=(c == 0),
                                    stop=(c == NCH - 1),
                                )
                            else:
                                nc.tensor.matmul(
                                    accs[j][:],
                                    w_sb[:, c * HD + j * 128 : c * HD + (j + 1) * 128],
                                    x_sb[:, c * HD : (c + 1) * HD],
                                    start=(c == 0),
                                    stop=(c == NCH - 1),
                                )

                # PSUM budget is 8 banks: tag "ps" [128,1024] bufs=2 (4 banks)
                # + "po" [128,512] bufs=2 (2) + "pd"/"wo" [128,512] bufs=1
                # (1 each).  Projections need 4 accumulators double-buffered
                # across n: two in "ps", one in "po", and the fourth
                # alternating between "pd" and "wo" by block parity.
                def new_accs(parity):
                    a = psp.tile([128, 2 * QB], F32, tag="ps", name="accA")
                    b = psp.tile([128, QB], F32, tag="po", name="accB")
                    c = psp.tile(
                        [128, QB], F32, tag=("pd" if parity == 0 else "wo"),
                        bufs=1, name="accC",
                    )
                    return [a[:, 0:QB], a[:, QB : 2 * QB], b[:], c[:]]

                def load_xblk(x_dram, n):
                    xblk = jp.tile(
                        [128, NCH * QB], BF16, tag="xblk", bufs=2, name="xblk"
                    )
                    nc.sync.dma_start(
                        out=xblk.rearrange("p (c q) -> p c q", q=QB),
                        in_=x_dram[:, n * QB : (n + 1) * QB].rearrange(
                            "(c p) q -> p c q", p=128
                        ),
                    )
                    return xblk

                for n in range(NQ):
                    xblk = xblk0 if n == 0 else load_xblk(kvT, n)
                    # K projection: hk^T[d, seq], stationary = wk chunk
                    accs = new_accs(0)
                    proj_block(xblk, w_sbs["wk"], accs, is_v=False)
                    for j in range(4):
                        nc.scalar.activation(
                            hk_sb[:, j * L + n * QB : j * L + (n + 1) * QB],
                            accs[j][:],
                            COPY,
                        )
                    # V projection: hv[k, 4h*128d], stationary = kv chunk cols
                    accs = new_accs(1)
                    proj_block(xblk, w_sbs["wv"], accs, is_v=True)
                    for j in range(4):
                        nc.scalar.activation(
                            hv_sb[:, (n * 4 + j) * HD : (n * 4 + j + 1) * HD],
                            accs[j][:],
                            COPY,
                        )
                for n in range(NQ):
                    xblk = load_xblk(qT, n)
                    accs = new_accs(n % 2)
                    proj_block(xblk, w_sbs["wq"], accs, is_v=False)
                    for j in range(4):
                        # last pass drains on DVE so ACT is free for the
                        # first attention exps (and psum slots free sooner)
                        if n == NQ - 1:
                            nc.vector.tensor_copy(
                                out=hq_sb[:, j * L + n * QB : j * L + (n + 1) * QB],
                                in_=accs[j][:],
                            )
                        else:
                            nc.scalar.activation(
                                hq_sb[:, j * L + n * QB : j * L + (n + 1) * QB],
                                accs[j][:],
                                COPY,
                            )

            # ---------------- attention ----------------
            with tc.tile_pool(name="attn", bufs=1) as ap:
                def flush(st):
                    # deferred normalization of the previous (n, h) iteration
                    h_, n_, ps_o_, ps_d_ = st
                    recip = ap.tile([128, QB], F32, tag="recip", bufs=2, name="recip")
                    nc.vector.reciprocal_approx_fast(out=recip[:], in_=ps_d_)
                    nc.vector.tensor_mul(
                        out=o_sb[:, h_ * L + n_ * QB : h_ * L + (n_ + 1) * QB],
                        in0=ps_o_,
                        in1=recip[:],
                    )

                # Flat software pipeline over the 16 (n, h) units: AV lags
                # scores by one kt-pair within a unit; the last AV pair, the
                # ones-matmul (partition reduction) and the normalization of
                # unit u are deferred into unit u+1 so the PE never waits on
                # ACT/DVE at a unit boundary.  Attention is ACT-bound (8 exps
                # x ~1.1us/unit vs ~7.1us of PE work), so Wo output-projection
                # groups are interleaved into the spare PE slots as soon as
                # their q-block is normalized; only block 3's Wo remains as a
                # short tail.
                units = [(n, h) for n in range(NQ) for h in range(HL)]
                state = {}  # u -> dict with ps_o, ps_d, exp_half, d128, (n, h)
                prev_flush = None
                wo_queue = []  # ready (qt, m) output groups
                wo_stage_eng = [0]  # alternate DVE for interleaved drains

                def emit_av(u, pair):
                    st = state[u]
                    n_, h_ = st["nh"]
                    for t in range(2):
                        kt = 2 * pair + t
                        e_sl = st["exp_half"][kt // 8][
                            :, (kt % 8) * QB : (kt % 8 + 1) * QB
                        ]
                        nc.tensor.matmul(
                            st["ps_o"][:],
                            hv_sb[:, kt * HD + h_ * 128 : kt * HD + (h_ + 1) * 128],
                            e_sl,
                            start=(kt == 0),
                            stop=(kt == NKT - 1),
                        )

                def emit_wo_group(qt, m, tag, on_act):
                    acc = psp.tile(
                        [128, QB], F32, tag=tag, bufs=(1 if tag == "wo" else 2),
                        name="woacc",
                    )
                    for h_ in range(HL):
                        nc.tensor.matmul(
                            acc[:],
                            o_sb[:, h_ * L + qt * 128 : h_ * L + (qt + 1) * 128],
                            wo_sb[:, h_ * DOUT + m * QB : h_ * DOUT + (m + 1) * QB],
                            start=(h_ == 0),
                            stop=(h_ == HL - 1),
                        )
                    stage = ap.tile([128, QB], BF16, tag="wstage", bufs=4, name="ws")
                    if on_act:
                        nc.scalar.activation(stage[:], acc[:], COPY)
                    else:
                        nc.vector.tensor_copy(out=stage[:], in_=acc[:])
                    nc.sync.dma_start(
                        out=out[qt * 128 : (qt + 1) * 128, m * QB : (m + 1) * QB],
                        in_=stage[:],
                    )

                for u, (n, h) in enumerate(units):
                    hq_sl = hq_sb[:, h * L + n * QB : h * L + (n + 1) * QB]
                    st = {
                        "nh": (n, h),
                        "ps_o": psp.tile([128, QB], F32, tag="po", name="ps_o"),
                        "ps_d": psp.tile([128, QB], F32, tag="pd", bufs=1, name="ps_d"),
                        "exp_half": [None, None],
                    }
                    state[u] = st
                    wo_emitted = 0
                    tl = {}  # denominator tree: level -> pending tiles
                    for p in range(8):
                        half = p // 4
                        if p % 4 == 0:
                            st["exp_half"][half] = ap.tile(
                                [128, 8 * QB], BF16, tag="exp", bufs=3, name="exp"
                            )
                        off = (p % 4) * 2 * QB
                        ps_s = psp.tile([128, 2 * QB], F32, tag="ps", name="ps_s")
                        for t in range(2):
                            kt = 2 * p + t
                            nc.tensor.matmul(
                                ps_s[:, t * QB : (t + 1) * QB],
                                hk_sb[:, h * L + kt * 128 : h * L + (kt + 1) * 128],
                                hq_sl,
                                start=True,
                                stop=True,
                            )
                        nc.scalar.activation(
                            st["exp_half"][half][:, off : off + 2 * QB], ps_s[:], EXP
                        )
                        # level-0 tree add over this exp pair, fold-up when ready
                        t0 = ap.tile([128, QB], BF16, tag="t0", bufs=2, name="t0")
                        nc.vector.tensor_add(
                            out=t0[:],
                            in0=st["exp_half"][half][:, off : off + QB],
                            in1=st["exp_half"][half][:, off + QB : off + 2 * QB],
                        )
                        tl.setdefault(0, []).append(t0)
                        lv = 0
                        while len(tl.get(lv, [])) == 2:
                            a, b = tl.pop(lv)
                            nxt = ap.tile(
                                [128, QB], BF16, tag=f"t{lv+1}", bufs=2,
                                name=f"t{lv+1}",
                            )
                            nc.vector.tensor_add(out=nxt[:], in0=a[:], in1=b[:])
                            tl.setdefault(lv + 1, []).append(nxt)
                            lv += 1
                        # deferred work from the previous unit / this unit
                        if p == 0 and u > 0:
                            emit_av(u - 1, 7)
                        if p >= 1:
                            emit_av(u, p - 1)
                        if p == 1 and u > 0:
                            pst = state[u - 1]
                            nc.tensor.matmul(
                                pst["ps_d"][:], ones_sb[:], pst["d128"][:],
                                start=True, stop=True,
                            )
                            n_, h_ = pst["nh"]
                            prev_flush = (h_, n_, pst["ps_o"], pst["ps_d"])
                        if p == 4 and prev_flush is not None:
                            fh, fn, fo, fd = prev_flush
                            flush(prev_flush)
                            prev_flush = None
                            if fh == HL - 1:
                                # block fn fully normalized: queue its Wo groups
                                wo_queue.extend(
                                    (fn * 4 + qq, m)
                                    for qq in range(4)
                                    for m in range(4)
                                )
                        if p >= 2 and wo_queue and wo_emitted < 6:
                            qt_, m_ = wo_queue.pop(0)
                            # drains alternate DVE/ACT to keep both under PE
                            emit_wo_group(qt_, m_, "wo", on_act=(wo_emitted % 2 == 1))
                            wo_emitted += 1
                    st["d128"] = tl[3][0]
                    if u > 0:
                        state.pop(u - 1)
                # drain the pipeline: last unit's AV tail, reduction, flushes
                last = len(units) - 1
                emit_av(last, 7)
                st = state[last]
                nc.tensor.matmul(
                    st["ps_d"][:], ones_sb[:], st["d128"][:], start=True, stop=True
                )
                if prev_flush is not None:  # unit last-1, ones'd at p1 of last
                    flush(prev_flush)
                n_, h_ = st["nh"]
                flush((h_, n_, st["ps_o"], st["ps_d"]))
                wo_queue.extend((3 * 4 + qq, m) for qq in range(4) for m in range(4))
                # Wo tail: remaining groups, psum banks alternating wo/po so
                # the next group's matmuls never wait on the previous drain.
                for i, (qt_, m_) in enumerate(wo_queue):
                    emit_wo_group(qt_, m_, "wo" if i % 2 == 0 else "po", on_act=True)
    nc.compile()
    return nc


def _get_nc():
    if "nc" not in _CACHE:
        _CACHE["nc"] = _build_nc()
    return _CACHE["nc"]


def make_in_maps(query, key_value, Wq, Wk, Wv, Wo):
    scale = 1.0 / math.sqrt(D)
    bf = ml_dtypes.bfloat16
    allones = np.ones((128, 128), bf)
    in_maps = []
    qT = [np.ascontiguousarray(query[b].T.astype(bf)) for b in range(B)]
    kvT = [np.ascontiguousarray(key_value[b].T.astype(bf)) for b in range(B)]
    for core in range(NC_):
        b, g = divmod(core, NC_ // B)
        sl = slice(g * HD, (g + 1) * HD)
        in_maps.append(
            {
                "qT": qT[b],
                "kvT": kvT[b],
                "wqT": np.ascontiguousarray((Wq[sl, :] * scale).T.astype(bf)),
                "wkT": np.ascontiguousarray(Wk[sl, :].T.astype(bf)),
                "wvT": np.ascontiguousarray(Wv[sl, :].T.astype(bf)),
                "woT": np.ascontiguousarray(Wo[:, sl].T.astype(bf)),
                "allones": allones,
            }
        )
    return in_maps


def _numpy_fallback(query, key_value, attention_mask, Wq, Wk, Wv, Wo):
    # Only reached if the mask is not all-ones (never per the problem spec).
    q64, kv64 = query.astype(np.float64), key_value.astype(np.float64)
    hq = (q64 @ Wq.T.astype(np.float64)).reshape(B, L, NH, D).transpose(0, 2, 1, 3)
    hk = (kv64 @ Wk.T.astype(np.float64)).reshape(B, L, NH, D).transpose(0, 2, 1, 3)
    hv = (kv64 @ Wv.T.astype(np.float64)).reshape(B, L, NH, D).transpose(0, 2, 1, 3)
    s = np.einsum("bhqd,bhkd->bhqk", hq, hk) / math.sqrt(D)
    mask = attention_mask[:, None, :, :]
    s = np.where(mask, s, -np.inf)
    s = s - s.max(axis=-1, keepdims=True)
    e = np.exp(s)
    p = e / np.maximum(e.sum(axis=-1, keepdims=True), 1e-300)
    p = np.where(mask, p, 0.0)
    o = np.einsum("bhqk,bhkd->bhqd", p, hv)
    o = o.transpose(0, 2, 1, 3).reshape(B, L, NH * D)
    return (o @ Wo.T.astype(np.float64)).astype(np.float32)


def kernel(query, key_value, attention_mask, Wq, Wk, Wv, Wo):
    query = np.asarray(query)
    key_value = np.asarray(key_value)
    attention_mask = np.asarray(attention_mask)
    Wq, Wk, Wv, Wo = (np.asarray(a) for a in (Wq, Wk, Wv, Wo))

    if not attention_mask.all():
        return _numpy_fallback(query, key_value, attention_mask, Wq, Wk, Wv, Wo)

    from concourse.bass_utils import run_bass_kernel_spmd

    nc = _get_nc()
    in_maps = make_in_maps(query, key_value, Wq, Wk, Wv, Wo)
    res = run_bass_kernel_spmd(nc, in_maps, list(range(NC_))).results
    out = np.zeros((B, L, DOUT), np.float32)
    for core in range(NC_):
        b = core // (NC_ // B)
        out[b] += res[core]["out"].astype(np.float32)
    return out


# revision 19
# speedup vs baseline: 1.3737x; 1.0059x over previous
"""Multi-head attention (B=2, L=2048, D=2048, 16 heads x 128) on 8 trn2 cores.

Sharding: tensor-parallel over heads (4 groups of 4 heads) x data-parallel
over batch (2) -> 8 cores.  Each core computes, for its (batch b, group g):
    hq = q_b @ Wq_g.T, hk = kv_b @ Wk_g.T, hv = kv_b @ Wv_g.T   (4 heads)
    per head: P = softmax(hq hk^T / sqrt(128)), o = P hv
    partial_out = concat_heads(o) @ Wo[:, g].T        [2048, 2048]
Host sums the 4 per-group partials for each batch.

All matmuls run in bf16 (fp32 PSUM accumulation).  bf16 stationary weight
loads are 1 cycle/row on the PE (vs ~4 for float32r), which removes most of
the LDWEIGHTS overhead that dominated the f32r version; bf16 also halves
DMA traffic.  End-to-end numeric error ~5e-3 (tolerance 2e-2).

The softmax denominator is computed off the PE: a bf16 pairwise tree of
DVE adds folds the 16 exp tiles into one [128, 512] partial, and a single
ones-matmul on the PE does the final cross-partition reduction.

Device layout (per core):
  xblk     [128, 16c x 512]   column block of qT/kvT, streamed per n
  hqT/hkT  [128 d, 4h x 2048 seq]  (d on partitions)
  hv       [128 k, 16 kt x 512(=4h x 128 d)]
  scores^T [128 k-tile, 2x512 q] in PSUM -> exp on ACT -> SBUF bf16
  AV:      o^T[128 d, 512 q] += hv_kt.T @ exp_kt  (PSUM accumulate)
  denom:   DVE bf16 tree -> d128; ones-matmul -> ps_d; recip+mul on DVE
  Wo:      out[128 q, 2048 dout] += o_chunk.T @ woT_chunk, per q-tile
"""
import math
import sys

for _p in ("/opt/trn_rl_repo", "/root/.axon_site/_ro/trn_rl_repo"):
    if _p not in sys.path:
        sys.path.append(_p)

import numpy as np
import ml_dtypes

B = 2
L = 2048           # LQ == LK
DIN = 2048
NH = 16            # total heads
HL = 4             # heads per core
D = 128            # head dim
HD = HL * D        # 512, head-group width
DOUT = 2048
NC_ = 8            # cores
NCH = DIN // 128   # 16 contraction chunks
NQ = 4             # q blocks of 512
QB = 512
NKT = L // 128     # 16 key tiles

_CACHE = {}


def _build_nc():
    import concourse.bacc as bacc
    import concourse.mybir as mybir
    import concourse.tile as tile

    BF16 = mybir.dt.bfloat16
    F32 = mybir.dt.float32

    nc = bacc.Bacc("TRN2", target_bir_lowering=False, debug=False)
    qT = nc.dram_tensor("qT", [DIN, L], BF16, kind="ExternalInput").ap()
    kvT = nc.dram_tensor("kvT", [DIN, L], BF16, kind="ExternalInput").ap()
    wqT = nc.dram_tensor("wqT", [DIN, HD], BF16, kind="ExternalInput").ap()
    wkT = nc.dram_tensor("wkT", [DIN, HD], BF16, kind="ExternalInput").ap()
    wvT = nc.dram_tensor("wvT", [DIN, HD], BF16, kind="ExternalInput").ap()
    woT = nc.dram_tensor("woT", [HD, DOUT], BF16, kind="ExternalInput").ap()
    allones = nc.dram_tensor("allones", [128, 128], BF16, kind="ExternalInput").ap()
    out = nc.dram_tensor("out", [L, DOUT], BF16, kind="ExternalOutput").ap()

    EXP = mybir.ActivationFunctionType.Exp
    COPY = mybir.ActivationFunctionType.Copy

    with tile.TileContext(nc) as tc:
        with (
            nc.allow_low_precision(reason="bf16 matmuls, ~5e-3 relmax vs 2e-2 tol"),
            tc.tile_pool(name="persist", bufs=1) as pp,
            tc.tile_pool(name="psum", bufs=2, space="PSUM") as psp,
        ):
            hq_sb = pp.tile([128, HL * L], BF16, tag="hq")
            hk_sb = pp.tile([128, HL * L], BF16, tag="hk")
            hv_sb = pp.tile([128, NKT * HD], BF16, tag="hv")
            o_sb = pp.tile([128, HL * L], BF16, tag="o")
            wo_sb = pp.tile([128, HL * DOUT], BF16, tag="wo")
            ones_sb = pp.tile([128, 128], BF16, tag="ones")
            nc.gpsimd.dma_start(out=ones_sb[:], in_=allones)

            # ---------------- projections ----------------
            with tc.tile_pool(name="proj", bufs=1) as jp:
                # wk and the first kv block gate the first matmul: stream both
                # in interleaved 4-chunk pieces so c=0 can start early.
                PIECES = [(0, 2), (2, 6), (6, 11), (11, 16)]

                def load_w_piece(w_sb, w_dram, piece):
                    c0, c1 = PIECES[piece]
                    nc.gpsimd.dma_start(
                        out=w_sb[:, c0 * HD : c1 * HD].rearrange(
                            "p (c m) -> p c m", m=HD
                        ),
                        in_=w_dram[c0 * 128 : c1 * 128, :].rearrange(
                            "(c p) m -> p c m", p=128
                        ),
                    )

                w_sbs = {
                    nm: jp.tile([128, NCH * HD], BF16, tag=f"w_{nm}", name=nm)
                    for nm in ("wk", "wv", "wq")
                }
                xblk0 = jp.tile([128, NCH * QB], BF16, tag="xblk", bufs=2, name="xblk")

                def load_x_piece(xblk, x_dram, n, piece):
                    c0, c1 = PIECES[piece]
                    nc.sync.dma_start(
                        out=xblk[:, c0 * QB : c1 * QB].rearrange(
                            "p (c q) -> p c q", q=QB
                        ),
                        in_=x_dram[c0 * 128 : c1 * 128, n * QB : (n + 1) * QB].rearrange(
                            "(c p) q -> p c q", p=128
                        ),
                    )

                for piece in range(4):
                    load_w_piece(w_sbs["wk"], wkT, piece)
                    load_x_piece(xblk0, kvT, 0, piece)
                for piece in range(4):
                    load_w_piece(w_sbs["wv"], wvT, piece)
                # wq/wo are not needed until the Q pass / attention: loaded
                # after the KV-pass emission so their packets don't compete
                # with the kv column blocks.

                def proj_block(x_sb, w_sb, accs, is_v):
                    for c in range(NCH):
                        for j in range(4):
                            if is_v:
                                nc.tensor.matmul(
                                    accs[j][:],
                                    x_sb[:, c * HD + j * 128 : c * HD + (j + 1) * 128],
                                    w_sb[:, c * HD : (c + 1) * HD],
                                    start=(c == 0),
                                    stop=(c == NCH - 1),
                                )
                            else:
                                nc.tensor.matmul(
                                    accs[j][:],
                                    w_sb[:, c * HD + j * 128 : c * HD + (j + 1) * 128],
                                    x_sb[:, c * HD : (c + 1) * HD],
                                    start=(c == 0),
                                    stop=(c == NCH - 1),
                                )

                # PSUM budget is 8 banks: tag "ps" [128,1024] bufs=2 (4 banks)
                # + "po" [128,512] bufs=2 (2) + "pd"/"wo" [128,512] bufs=1
                # (1 each).  Projections need 4 accumulators double-buffered
                # across n: two in "ps", one in "po", and the fourth
                # alternating between "pd" and "wo" by block parity.
                def new_accs(parity):
                    a = psp.tile([128, 2 * QB], F32, tag="ps", name="accA")
                    b = psp.tile([128, QB], F32, tag="po", name="accB")
                    c = psp.tile(
                        [128, QB], F32, tag=("pd" if parity == 0 else "wo"),
                        bufs=1, name="accC",
                    )
                    return [a[:, 0:QB], a[:, QB : 2 * QB], b[:], c[:]]

                def load_xblk(x_dram, n):
                    xblk = jp.tile(
                        [128, NCH * QB], BF16, tag="xblk", bufs=2, name="xblk"
                    )
                    nc.sync.dma_start(
                        out=xblk.rearrange("p (c q) -> p c q", q=QB),
                        in_=x_dram[:, n * QB : (n + 1) * QB].rearrange(
                            "(c p) q -> p c q", p=128
                        ),
                    )
                    return xblk

                for n in range(NQ):
                    xblk = xblk0 if n == 0 else load_xblk(kvT, n)
                    # K projection: hk^T[d, seq], stationary = wk chunk
                    accs = new_accs(0)
                    proj_block(xblk, w_sbs["wk"], accs, is_v=False)
                    for j in range(4):
                        nc.scalar.activation(
                            hk_sb[:, j * L + n * QB : j * L + (n + 1) * QB],
                            accs[j][:],
                            COPY,
                        )
                    # V projection: hv[k, 4h*128d], stationary = kv chunk cols
                    accs = new_accs(1)
                    proj_block(xblk, w_sbs["wv"], accs, is_v=True)
                    for j in range(4):
                        nc.scalar.activation(
                            hv_sb[:, (n * 4 + j) * HD : (n * 4 + j + 1) * HD],
                            accs[j][:],
                            COPY,
                        )
                for piece in range(4):
                    load_w_piece(w_sbs["wq"], wqT, piece)
                for h in range(HL):
                    nc.gpsimd.dma_start(
                        out=wo_sb[:, h * DOUT : (h + 1) * DOUT],
                        in_=woT[h * 128 : (h + 1) * 128, :],
                    )
                for n in range(NQ):
                    xblk = load_xblk(qT, n)
                    accs = new_accs(n % 2)
                    proj_block(xblk, w_sbs["wq"], accs, is_v=False)
                    for j in range(4):
                        # last pass drains on DVE so ACT is free for the
                        # first attention exps (and psum slots free sooner)
                        if n == NQ - 1:
                            nc.vector.tensor_copy(
                                out=hq_sb[:, j * L + n * QB : j * L + (n + 1) * QB],
                                in_=accs[j][:],
                            )
                        else:
                            nc.scalar.activation(
                                hq_sb[:, j * L + n * QB : j * L + (n + 1) * QB],
                                accs[j][:],
                                COPY,
                            )

            # ---------------- attention ----------------
            with tc.tile_pool(name="attn", bufs=1) as ap:
                def flush(st):
                    # deferred normalization of the previous (n, h) iteration
                    h_, n_, ps_o_, ps_d_ = st
                    recip = ap.tile([128, QB], F32, tag="recip", bufs=2, name="recip")
                    nc.vector.reciprocal_approx_fast(out=recip[:], in_=ps_d_)
                    nc.vector.tensor_mul(
                        out=o_sb[:, h_ * L + n_ * QB : h_ * L + (n_ + 1) * QB],
                        in0=ps_o_,
                        in1=recip[:],
                    )

                # Flat software pipeline over the 16 (n, h) units: AV lags
                # scores by one kt-pair within a unit; the last AV pair, the
                # ones-matmul (partition reduction) and the normalization of
                # unit u are deferred into unit u+1 so the PE never waits on
                # ACT/DVE at a unit boundary.  Attention is ACT-bound (8 exps
                # x ~1.1us/unit vs ~7.1us of PE work), so Wo output-projection
                # groups are interleaved into the spare PE slots as soon as
                # their q-block is normalized; only block 3's Wo remains as a
                # short tail.
                units = [(n, h) for n in range(NQ) for h in range(HL)]
                state = {}  # u -> dict with ps_o, ps_d, exp_half, d128, (n, h)
                prev_flush = None
                wo_queue = []  # ready (qt, m) output groups
                wo_stage_eng = [0]  # alternate DVE for interleaved drains

                def emit_av(u, pair):
                    st = state[u]
                    n_, h_ = st["nh"]
                    for t in range(2):
                        kt = 2 * pair + t
                        e_sl = st["exp_half"][kt // 8][
                            :, (kt % 8) * QB : (kt % 8 + 1) * QB
                        ]
                        nc.tensor.matmul(
                            st["ps_o"][:],
                            hv_sb[:, kt * HD + h_ * 128 : kt * HD + (h_ + 1) * 128],
                            e_sl,
                            start=(kt == 0),
                            stop=(kt == NKT - 1),
                        )

                def emit_wo_group(qt, m, tag, on_act):
                    acc = psp.tile(
                        [128, QB], F32, tag=tag,
                        bufs=(1 if tag in ("wo", "pd") else 2),
                        name="woacc",
                    )
                    for h_ in range(HL):
                        nc.tensor.matmul(
                            acc[:],
                            o_sb[:, h_ * L + qt * 128 : h_ * L + (qt + 1) * 128],
                            wo_sb[:, h_ * DOUT + m * QB : h_ * DOUT + (m + 1) * QB],
                            start=(h_ == 0),
                            stop=(h_ == HL - 1),
                        )
                    stage = ap.tile([128, QB], BF16, tag="wstage", bufs=4, name="ws")
                    if on_act:
                        nc.scalar.activation(stage[:], acc[:], COPY)
                    else:
                        nc.vector.tensor_copy(out=stage[:], in_=acc[:])
                    nc.sync.dma_start(
                        out=out[qt * 128 : (qt + 1) * 128, m * QB : (m + 1) * QB],
                        in_=stage[:],
                    )

                for u, (n, h) in enumerate(units):
                    hq_sl = hq_sb[:, h * L + n * QB : h * L + (n + 1) * QB]
                    st = {
                        "nh": (n, h),
                        "ps_o": psp.tile([128, QB], F32, tag="po", name="ps_o"),
                        "ps_d": psp.tile([128, QB], F32, tag="pd", bufs=1, name="ps_d"),
                        "exp_half": [None, None],
                    }
                    state[u] = st
                    wo_emitted = 0
                    tl = {}  # denominator tree: level -> pending tiles
                    for p in range(8):
                        half = p // 4
                        if p % 4 == 0:
                            st["exp_half"][half] = ap.tile(
                                [128, 8 * QB], BF16, tag="exp", bufs=3, name="exp"
                            )
                        off = (p % 4) * 2 * QB
                        ps_s = psp.tile([128, 2 * QB], F32, tag="ps", name="ps_s")
                        for t in range(2):
                            kt = 2 * p + t
                            nc.tensor.matmul(
                                ps_s[:, t * QB : (t + 1) * QB],
                                hk_sb[:, h * L + kt * 128 : h * L + (kt + 1) * 128],
                                hq_sl,
                                start=True,
                                stop=True,
                            )
                        nc.scalar.activation(
                            st["exp_half"][half][:, off : off + 2 * QB], ps_s[:], EXP
                        )
                        # level-0 tree add over this exp pair, fold-up when ready
                        t0 = ap.tile([128, QB], BF16, tag="t0", bufs=2, name="t0")
                        nc.vector.tensor_add(
                            out=t0[:],
                            in0=st["exp_half"][half][:, off : off + QB],
                            in1=st["exp_half"][half][:, off + QB : off + 2 * QB],
                        )
                        tl.setdefault(0, []).append(t0)
                        lv = 0
                        while len(tl.get(lv, [])) == 2:
                            a, b = tl.pop(lv)
                            nxt = ap.tile(
                                [128, QB], BF16, tag=f"t{lv+1}", bufs=2,
                                name=f"t{lv+1}",
                            )
                            nc.vector.tensor_add(out=nxt[:], in0=a[:], in1=b[:])
                            tl.setdefault(lv + 1, []).append(nxt)
                            lv += 1
                        # deferred work from the previous unit / this unit
                        if p == 0 and u > 0:
                            emit_av(u - 1, 7)
                        if p >= 1:
                            emit_av(u, p - 1)
                        if p == 1 and u > 0:
                            pst = state[u - 1]
                            nc.tensor.matmul(
                                pst["ps_d"][:], ones_sb[:], pst["d128"][:],
                                start=True, stop=True,
                            )
                            n_, h_ = pst["nh"]
                            prev_flush = (h_, n_, pst["ps_o"], pst["ps_d"])
                        if p == 4 and prev_flush is not None:
                            fh, fn, fo, fd = prev_flush
                            flush(prev_flush)
                            prev_flush = None
                            if fh == HL - 1:
                                # block fn fully normalized: queue its Wo groups
                                wo_queue.extend(
                                    (fn * 4 + qq, m)
                                    for qq in range(4)
                                    for m in range(4)
                                )
                        if p >= 2 and wo_queue and wo_emitted < 6:
                            qt_, m_ = wo_queue.pop(0)
                            # drains alternate DVE/ACT to keep both under PE
                            emit_wo_group(qt_, m_, "wo", on_act=(wo_emitted % 2 == 1))
                            wo_emitted += 1
                    st["d128"] = tl[3][0]
                    if u > 0:
                        state.pop(u - 1)
                # drain the pipeline: last unit's AV tail, reduction, flushes
                last = len(units) - 1
                emit_av(last, 7)
                st = state[last]
                nc.tensor.matmul(
                    st["ps_d"][:], ones_sb[:], st["d128"][:], start=True, stop=True
                )
                if prev_flush is not None:  # unit last-1, ones'd at p1 of last
                    flush(prev_flush)
                n_, h_ = st["nh"]
                flush((h_, n_, st["ps_o"], st["ps_d"]))
                wo_queue.extend((3 * 4 + qq, m) for qq in range(4) for m in range(4))
                # Wo tail: remaining groups rotate over three free psum banks
                # so the next group's matmuls never wait on a pending drain.
                tail_tags = ["wo", "po", "pd"]
                for i, (qt_, m_) in enumerate(wo_queue):
                    emit_wo_group(qt_, m_, tail_tags[i % 3], on_act=(i % 2 == 0))
    nc.compile()
    return nc


def _get_nc():
    if "nc" not in _CACHE:
        _CACHE["nc"] = _build_nc()
    return _CACHE["nc"]


def make_in_maps(query, key_value, Wq, Wk, Wv, Wo):
    scale = 1.0 / math.sqrt(D)
    bf = ml_dtypes.bfloat16
    allones = np.ones((128, 128), bf)
    in_maps = []
    qT = [np.ascontiguousarray(query[b].T.astype(bf)) for b in range(B)]
    kvT = [np.ascontiguousarray(key_value[b].T.astype(bf)) for b in range(B)]
    for core in range(NC_):
        b, g = divmod(core, NC_ // B)
        sl = slice(g * HD, (g + 1) * HD)
        in_maps.append(
            {
                "qT": qT[b],
                "kvT": kvT[b],
                "wqT": np.ascontiguousarray((Wq[sl, :] * scale).T.astype(bf)),
                "wkT": np.ascontiguousarray(Wk[sl, :].T.astype(bf)),
                "wvT": np.ascontiguousarray(Wv[sl, :].T.astype(bf)),
                "woT": np.ascontiguousarray(Wo[:, sl].T.astype(bf)),
                "allones": allones,
            }
        )
    return in_maps


def _numpy_fallback(query, key_value, attention_mask, Wq, Wk, Wv, Wo):
    # Only reached if the mask is not all-ones (never per the problem spec).
    q64, kv64 = query.astype(np.float64), key_value.astype(np.float64)
    hq = (q64 @ Wq.T.astype(np.float64)).reshape(B, L, NH, D).transpose(0, 2, 1, 3)
    hk = (kv64 @ Wk.T.astype(np.float64)).reshape(B, L, NH, D).transpose(0, 2, 1, 3)
    hv = (kv64 @ Wv.T.astype(np.float64)).reshape(B, L, NH, D).transpose(0, 2, 1, 3)
    s = np.einsum("bhqd,bhkd->bhqk", hq, hk) / math.sqrt(D)
    mask = attention_mask[:, None, :, :]
    s = np.where(mask, s, -np.inf)
    s = s - s.max(axis=-1, keepdims=True)
    e = np.exp(s)
    p = e / np.maximum(e.sum(axis=-1, keepdims=True), 1e-300)
    p = np.where(mask, p, 0.0)
    o = np.einsum("bhqk,bhkd->bhqd", p, hv)
    o = o.transpose(0, 2, 1, 3).reshape(B, L, NH * D)
    return (o @ Wo.T.astype(np.float64)).astype(np.float32)


def kernel(query, key_value, attention_mask, Wq, Wk, Wv, Wo):
    query = np.asarray(query)
    key_value = np.asarray(key_value)
    attention_mask = np.asarray(attention_mask)
    Wq, Wk, Wv, Wo = (np.asarray(a) for a in (Wq, Wk, Wv, Wo))

    if not attention_mask.all():
        return _numpy_fallback(query, key_value, attention_mask, Wq, Wk, Wv, Wo)

    from concourse.bass_utils import run_bass_kernel_spmd

    nc = _get_nc()
    in_maps = make_in_maps(query, key_value, Wq, Wk, Wv, Wo)
    res = run_bass_kernel_spmd(nc, in_maps, list(range(NC_))).results
    out = np.zeros((B, L, DOUT), np.float32)
    for core in range(NC_):
        b = core // (NC_ // B)
        out[b] += res[core]["out"].astype(np.float32)
    return out


# revision 23
# speedup vs baseline: 1.3941x; 1.0148x over previous
"""Multi-head attention (B=2, L=2048, D=2048, 16 heads x 128) on 8 trn2 cores.

Sharding: tensor-parallel over heads (4 groups of 4 heads) x data-parallel
over batch (2) -> 8 cores.  Each core computes, for its (batch b, group g):
    hq = q_b @ Wq_g.T, hk = kv_b @ Wk_g.T, hv = kv_b @ Wv_g.T   (4 heads)
    per head: P = softmax(hq hk^T / sqrt(128)), o = P hv
    partial_out = concat_heads(o) @ Wo[:, g].T        [2048, 2048]
Host sums the 4 per-group partials for each batch.

All matmuls run in bf16 (fp32 PSUM accumulation).  bf16 stationary weight
loads are 1 cycle/row on the PE (vs ~4 for float32r), which removes most of
the LDWEIGHTS overhead that dominated the f32r version; bf16 also halves
DMA traffic.  End-to-end numeric error ~5e-3 (tolerance 2e-2).

The softmax denominator is computed off the PE: a bf16 pairwise tree of
DVE adds folds the 16 exp tiles into one [128, 512] partial, and a single
ones-matmul on the PE does the final cross-partition reduction.

Device layout (per core):
  xblk     [128, 16c x 512]   column block of qT/kvT, streamed per n
  hqT/hkT  [128 d, 4h x 2048 seq]  (d on partitions)
  hv       [128 k, 16 kt x 512(=4h x 128 d)]
  scores^T [128 k-tile, 2x512 q] in PSUM -> exp on ACT -> SBUF bf16
  AV:      o^T[128 d, 512 q] += hv_kt.T @ exp_kt  (PSUM accumulate)
  denom:   DVE bf16 tree -> d128; ones-matmul -> ps_d; recip+mul on DVE
  Wo:      out[128 q, 2048 dout] += o_chunk.T @ woT_chunk, per q-tile
"""
import math
import sys

for _p in ("/opt/trn_rl_repo", "/root/.axon_site/_ro/trn_rl_repo"):
    if _p not in sys.path:
        sys.path.append(_p)

import numpy as np
import ml_dtypes

B = 2
L = 2048           # LQ == LK
DIN = 2048
NH = 16            # total heads
HL = 4             # heads per core
D = 128            # head dim
HD = HL * D        # 512, head-group width
DOUT = 2048
NC_ = 8            # cores
NCH = DIN // 128   # 16 contraction chunks
NQ = 4             # q blocks of 512
QB = 512
NKT = L // 128     # 16 key tiles

_CACHE = {}


def _build_nc():
    import concourse.bacc as bacc
    import concourse.mybir as mybir
    import concourse.tile as tile

    BF16 = mybir.dt.bfloat16
    F32 = mybir.dt.float32

    nc = bacc.Bacc("TRN2", target_bir_lowering=False, debug=False)
    qT = nc.dram_tensor("qT", [DIN, L], BF16, kind="ExternalInput").ap()
    kvT = nc.dram_tensor("kvT", [DIN, L], BF16, kind="ExternalInput").ap()
    wqT = nc.dram_tensor("wqT", [DIN, HD], BF16, kind="ExternalInput").ap()
    wkT = nc.dram_tensor("wkT", [DIN, HD], BF16, kind="ExternalInput").ap()
    wvT = nc.dram_tensor("wvT", [DIN, HD], BF16, kind="ExternalInput").ap()
    woT = nc.dram_tensor("woT", [HD, DOUT], BF16, kind="ExternalInput").ap()
    allones = nc.dram_tensor("allones", [128, 128], BF16, kind="ExternalInput").ap()
    out = nc.dram_tensor("out", [L, DOUT], BF16, kind="ExternalOutput").ap()

    EXP = mybir.ActivationFunctionType.Exp
    COPY = mybir.ActivationFunctionType.Copy

    with tile.TileContext(nc) as tc:
        with (
            nc.allow_low_precision(reason="bf16 matmuls, ~5e-3 relmax vs 2e-2 tol"),
            tc.tile_pool(name="persist", bufs=1) as pp,
            tc.tile_pool(name="psum", bufs=2, space="PSUM") as psp,
        ):
            hq_sb = pp.tile([128, HL * L], BF16, tag="hq")
            hk_sb = pp.tile([128, HL * L], BF16, tag="hk")
            hv_sb = pp.tile([128, NKT * HD], BF16, tag="hv")
            o_sb = pp.tile([128, HL * L], BF16, tag="o")
            wo_sb = pp.tile([128, HL * DOUT], BF16, tag="wo")
            ones_sb = pp.tile([128, 128], BF16, tag="ones")
            nc.gpsimd.dma_start(out=ones_sb[:], in_=allones)

            # ---------------- projections ----------------
            with tc.tile_pool(name="proj", bufs=1) as jp:
                # wk and the first kv block gate the first matmul: stream both
                # in interleaved 4-chunk pieces so c=0 can start early.
                PIECES = [(0, 2), (2, 6), (6, 11), (11, 16)]

                def load_w_piece(w_sb, w_dram, piece):
                    c0, c1 = PIECES[piece]
                    nc.gpsimd.dma_start(
                        out=w_sb[:, c0 * HD : c1 * HD].rearrange(
                            "p (c m) -> p c m", m=HD
                        ),
                        in_=w_dram[c0 * 128 : c1 * 128, :].rearrange(
                            "(c p) m -> p c m", p=128
                        ),
                    )

                w_sbs = {
                    nm: jp.tile([128, NCH * HD], BF16, tag=f"w_{nm}", name=nm)
                    for nm in ("wk", "wv", "wq")
                }
                xblk0 = jp.tile([128, NCH * QB], BF16, tag="xblk", bufs=3, name="xblk")

                def load_x_piece(xblk, x_dram, n, piece):
                    c0, c1 = PIECES[piece]
                    nc.sync.dma_start(
                        out=xblk[:, c0 * QB : c1 * QB].rearrange(
                            "p (c q) -> p c q", q=QB
                        ),
                        in_=x_dram[c0 * 128 : c1 * 128, n * QB : (n + 1) * QB].rearrange(
                            "(c p) q -> p c q", p=128
                        ),
                    )

                for piece in range(4):
                    load_w_piece(w_sbs["wk"], wkT, piece)
                    load_x_piece(xblk0, kvT, 0, piece)
                for piece in range(4):
                    load_w_piece(w_sbs["wv"], wvT, piece)
                # wq/wo are not needed until the Q pass / attention: loaded
                # after the KV-pass emission so their packets don't compete
                # with the kv column blocks.

                def proj_block(x_sb, w_sb, accs, is_v):
                    for c in range(NCH):
                        for j in range(4):
                            if is_v:
                                nc.tensor.matmul(
                                    accs[j][:],
                                    x_sb[:, c * HD + j * 128 : c * HD + (j + 1) * 128],
                                    w_sb[:, c * HD : (c + 1) * HD],
                                    start=(c == 0),
                                    stop=(c == NCH - 1),
                                )
                            else:
                                nc.tensor.matmul(
                                    accs[j][:],
                                    w_sb[:, c * HD + j * 128 : c * HD + (j + 1) * 128],
                                    x_sb[:, c * HD : (c + 1) * HD],
                                    start=(c == 0),
                                    stop=(c == NCH - 1),
                                )

                # PSUM budget is 8 banks: tag "ps" [128,1024] bufs=2 (4 banks)
                # + "po" [128,512] bufs=2 (2) + "pd"/"wo" [128,512] bufs=1
                # (1 each).  Projections need 4 accumulators double-buffered
                # across n: two in "ps", one in "po", and the fourth
                # alternating between "pd" and "wo" by block parity.
                def new_accs(parity):
                    a = psp.tile([128, 2 * QB], F32, tag="ps", name="accA")
                    b = psp.tile([128, QB], F32, tag="po", name="accB")
                    c = psp.tile(
                        [128, QB], F32, tag=("pd" if parity == 0 else "wo"),
                        bufs=1, name="accC",
                    )
                    return [a[:, 0:QB], a[:, QB : 2 * QB], b[:], c[:]]

                def load_xblk(x_dram, n):
                    xblk = jp.tile(
                        [128, NCH * QB], BF16, tag="xblk", bufs=3, name="xblk"
                    )
                    nc.sync.dma_start(
                        out=xblk.rearrange("p (c q) -> p c q", q=QB),
                        in_=x_dram[:, n * QB : (n + 1) * QB].rearrange(
                            "(c p) q -> p c q", p=128
                        ),
                    )
                    return xblk

                for n in range(NQ):
                    xblk = xblk0 if n == 0 else load_xblk(kvT, n)
                    # K projection: hk^T[d, seq], stationary = wk chunk
                    accs = new_accs(0)
                    proj_block(xblk, w_sbs["wk"], accs, is_v=False)
                    for j in range(4):
                        nc.scalar.activation(
                            hk_sb[:, j * L + n * QB : j * L + (n + 1) * QB],
                            accs[j][:],
                            COPY,
                        )
                    # V projection: hv[k, 4h*128d], stationary = kv chunk cols
                    accs = new_accs(1)
                    proj_block(xblk, w_sbs["wv"], accs, is_v=True)
                    for j in range(4):
                        nc.scalar.activation(
                            hv_sb[:, (n * 4 + j) * HD : (n * 4 + j + 1) * HD],
                            accs[j][:],
                            COPY,
                        )
                for piece in range(4):
                    load_w_piece(w_sbs["wq"], wqT, piece)
                for h in range(HL):
                    nc.gpsimd.dma_start(
                        out=wo_sb[:, h * DOUT : (h + 1) * DOUT],
                        in_=woT[h * 128 : (h + 1) * 128, :],
                    )
                for n in range(NQ):
                    xblk = load_xblk(qT, n)
                    accs = new_accs(n % 2)
                    proj_block(xblk, w_sbs["wq"], accs, is_v=False)
                    for j in range(4):
                        # last pass drains on DVE so ACT is free for the
                        # first attention exps (and psum slots free sooner)
                        if n == NQ - 1:
                            nc.vector.tensor_copy(
                                out=hq_sb[:, j * L + n * QB : j * L + (n + 1) * QB],
                                in_=accs[j][:],
                            )
                        else:
                            nc.scalar.activation(
                                hq_sb[:, j * L + n * QB : j * L + (n + 1) * QB],
                                accs[j][:],
                                COPY,
                            )

            # ---------------- attention ----------------
            with tc.tile_pool(name="attn", bufs=1) as ap:
                def flush(st):
                    # deferred normalization of the previous (n, h) iteration
                    h_, n_, ps_o_, ps_d_ = st
                    recip = ap.tile([128, QB], F32, tag="recip", bufs=2, name="recip")
                    nc.vector.reciprocal_approx_fast(out=recip[:], in_=ps_d_)
                    nc.vector.tensor_mul(
                        out=o_sb[:, h_ * L + n_ * QB : h_ * L + (n_ + 1) * QB],
                        in0=ps_o_,
                        in1=recip[:],
                    )

                # Flat software pipeline over the 16 (n, h) units: AV lags
                # scores by one kt-pair within a unit; the last AV pair, the
                # ones-matmul (partition reduction) and the normalization of
                # unit u are deferred into unit u+1 so the PE never waits on
                # ACT/DVE at a unit boundary.  Attention is ACT-bound (8 exps
                # x ~1.1us/unit vs ~7.1us of PE work), so Wo output-projection
                # groups are interleaved into the spare PE slots as soon as
                # their q-block is normalized; only block 3's Wo remains as a
                # short tail.
                units = [(n, h) for n in range(NQ) for h in range(HL)]
                state = {}  # u -> dict with ps_o, ps_d, exp_half, d128, (n, h)
                prev_flush = None
                wo_queue = []  # ready (qt, m) output groups
                wo_stage_eng = [0]  # alternate DVE for interleaved drains

                def emit_av(u, pair):
                    st = state[u]
                    n_, h_ = st["nh"]
                    for t in range(2):
                        kt = 2 * pair + t
                        e_sl = st["exp_half"][kt // 8][
                            :, (kt % 8) * QB : (kt % 8 + 1) * QB
                        ]
                        nc.tensor.matmul(
                            st["ps_o"][:],
                            hv_sb[:, kt * HD + h_ * 128 : kt * HD + (h_ + 1) * 128],
                            e_sl,
                            start=(kt == 0),
                            stop=(kt == NKT - 1),
                        )

                def emit_wo_group(qt, m, tag, on_act):
                    acc = psp.tile(
                        [128, QB], F32, tag=tag,
                        bufs=(1 if tag in ("wo", "pd") else 2),
                        name="woacc",
                    )
                    for h_ in range(HL):
                        nc.tensor.matmul(
                            acc[:],
                            o_sb[:, h_ * L + qt * 128 : h_ * L + (qt + 1) * 128],
                            wo_sb[:, h_ * DOUT + m * QB : h_ * DOUT + (m + 1) * QB],
                            start=(h_ == 0),
                            stop=(h_ == HL - 1),
                        )
                    stage = ap.tile([128, QB], BF16, tag="wstage", bufs=4, name="ws")
                    if on_act:
                        nc.scalar.activation(stage[:], acc[:], COPY)
                    else:
                        nc.vector.tensor_copy(out=stage[:], in_=acc[:])
                    nc.sync.dma_start(
                        out=out[qt * 128 : (qt + 1) * 128, m * QB : (m + 1) * QB],
                        in_=stage[:],
                    )

                for u, (n, h) in enumerate(units):
                    hq_sl = hq_sb[:, h * L + n * QB : h * L + (n + 1) * QB]
                    st = {
                        "nh": (n, h),
                        "ps_o": psp.tile([128, QB], F32, tag="po", name="ps_o"),
                        "ps_d": psp.tile([128, QB], F32, tag="pd", bufs=1, name="ps_d"),
                        "exp_half": [None, None],
                    }
                    state[u] = st
                    wo_emitted = 0
                    tl = {}  # denominator tree: level -> pending tiles
                    for p in range(8):
                        half = p // 4
                        if p % 4 == 0:
                            st["exp_half"][half] = ap.tile(
                                [128, 8 * QB], BF16, tag="exp", bufs=3, name="exp"
                            )
                        off = (p % 4) * 2 * QB
                        ps_s = psp.tile([128, 2 * QB], F32, tag="ps", name="ps_s")
                        for t in range(2):
                            kt = 2 * p + t
                            nc.tensor.matmul(
                                ps_s[:, t * QB : (t + 1) * QB],
                                hk_sb[:, h * L + kt * 128 : h * L + (kt + 1) * 128],
                                hq_sl,
                                start=True,
                                stop=True,
                            )
                        nc.scalar.activation(
                            st["exp_half"][half][:, off : off + 2 * QB], ps_s[:], EXP
                        )
                        # level-0 tree add over this exp pair, fold-up when ready
                        t0 = ap.tile([128, QB], BF16, tag="t0", bufs=2, name="t0")
                        nc.vector.tensor_add(
                            out=t0[:],
                            in0=st["exp_half"][half][:, off : off + QB],
                            in1=st["exp_half"][half][:, off + QB : off + 2 * QB],
                        )
                        tl.setdefault(0, []).append(t0)
                        lv = 0
                        while len(tl.get(lv, [])) == 2:
                            a, b = tl.pop(lv)
                            nxt = ap.tile(
                                [128, QB], BF16, tag=f"t{lv+1}", bufs=2,
                                name=f"t{lv+1}",
                            )
                            nc.vector.tensor_add(out=nxt[:], in0=a[:], in1=b[:])
                            tl.setdefault(lv + 1, []).append(nxt)
                            lv += 1
                        # deferred work from the previous unit / this unit
                        if p == 0 and u > 0:
                            emit_av(u - 1, 7)
                        if p >= 1:
                            emit_av(u, p - 1)
                        if p == 1 and u > 0:
                            pst = state[u - 1]
                            nc.tensor.matmul(
                                pst["ps_d"][:], ones_sb[:], pst["d128"][:],
                                start=True, stop=True,
                            )
                            n_, h_ = pst["nh"]
                            prev_flush = (h_, n_, pst["ps_o"], pst["ps_d"])
                        if p == 4 and prev_flush is not None:
                            fh, fn, fo, fd = prev_flush
                            flush(prev_flush)
                            prev_flush = None
                            if fh == HL - 1:
                                # block fn fully normalized: queue its Wo groups
                                wo_queue.extend(
                                    (fn * 4 + qq, m)
                                    for qq in range(4)
                                    for m in range(4)
                                )
                        if p >= 2 and p != 4 and wo_queue and wo_emitted < 6:
                            qt_, m_ = wo_queue.pop(0)
                            # drains alternate DVE/ACT to keep both under PE
                            emit_wo_group(qt_, m_, "wo", on_act=(wo_emitted % 2 == 1))
                            wo_emitted += 1
                    st["d128"] = tl[3][0]
                    if u > 0:
                        state.pop(u - 1)
                # drain the pipeline: last unit's AV tail, reduction, flushes
                last = len(units) - 1
                emit_av(last, 7)
                st = state[last]
                nc.tensor.matmul(
                    st["ps_d"][:], ones_sb[:], st["d128"][:], start=True, stop=True
                )
                if prev_flush is not None:  # unit last-1, ones'd at p1 of last
                    flush(prev_flush)
                n_, h_ = st["nh"]
                flush((h_, n_, st["ps_o"], st["ps_d"]))
                wo_queue.extend((3 * 4 + qq, m) for qq in range(4) for m in range(4))
                # Wo tail: remaining groups rotate over three free psum banks
                # so the next group's matmuls never wait on a pending drain.
                # The first three groups' h=0..2 matmuls are emitted before any
                # h=3 matmul so the PE stays busy while the final flush (which
                # produces o for h=3) drains on DVE.
                tail_tags = ["wo", "po", "pd"]

                def wo_mm(acc, qt_, m_, h_):
                    nc.tensor.matmul(
                        acc[:],
                        o_sb[:, h_ * L + qt_ * 128 : h_ * L + (qt_ + 1) * 128],
                        wo_sb[:, h_ * DOUT + m_ * QB : h_ * DOUT + (m_ + 1) * QB],
                        start=(h_ == 0),
                        stop=(h_ == HL - 1),
                    )

                head = wo_queue[:3]
                head_accs = []
                for i, (qt_, m_) in enumerate(head):
                    acc = psp.tile(
                        [128, QB], F32, tag=tail_tags[i],
                        bufs=(1 if tail_tags[i] in ("wo", "pd") else 2),
                        name="woacc",
                    )
                    head_accs.append(acc)
                    for h_ in range(HL - 1):
                        wo_mm(acc, qt_, m_, h_)
                for i, (qt_, m_) in enumerate(head):
                    acc = head_accs[i]
                    wo_mm(acc, qt_, m_, HL - 1)
                    stage = ap.tile([128, QB], BF16, tag="wstage", bufs=4, name="ws")
                    nc.scalar.activation(stage[:], acc[:], COPY)
                    nc.sync.dma_start(
                        out=out[qt_ * 128 : (qt_ + 1) * 128, m_ * QB : (m_ + 1) * QB],
                        in_=stage[:],
                    )
                for i, (qt_, m_) in enumerate(wo_queue[3:]):
                    emit_wo_group(qt_, m_, tail_tags[i % 3], on_act=(i % 2 == 0))
    nc.compile()
    return nc


def _get_nc():
    if "nc" not in _CACHE:
        _CACHE["nc"] = _build_nc()
    return _CACHE["nc"]


def make_in_maps(query, key_value, Wq, Wk, Wv, Wo):
    scale = 1.0 / math.sqrt(D)
    bf = ml_dtypes.bfloat16
    allones = np.ones((128, 128), bf)
    in_maps = []
    qT = [np.ascontiguousarray(query[b].T.astype(bf)) for b in range(B)]
    kvT = [np.ascontiguousarray(key_value[b].T.astype(bf)) for b in range(B)]
    for core in range(NC_):
        b, g = divmod(core, NC_ // B)
        sl = slice(g * HD, (g + 1) * HD)
        in_maps.append(
            {
                "qT": qT[b],
                "kvT": kvT[b],
                "wqT": np.ascontiguousarray((Wq[sl, :] * scale).T.astype(bf)),
                "wkT": np.ascontiguousarray(Wk[sl, :].T.astype(bf)),
                "wvT": np.ascontiguousarray(Wv[sl, :].T.astype(bf)),
                "woT": np.ascontiguousarray(Wo[:, sl].T.astype(bf)),
                "allones": allones,
            }
        )
    return in_maps


def _numpy_fallback(query, key_value, attention_mask, Wq, Wk, Wv, Wo):
    # Only reached if the mask is not all-ones (never per the problem spec).
    q64, kv64 = query.astype(np.float64), key_value.astype(np.float64)
    hq = (q64 @ Wq.T.astype(np.float64)).reshape(B, L, NH, D).transpose(0, 2, 1, 3)
    hk = (kv64 @ Wk.T.astype(np.float64)).reshape(B, L, NH, D).transpose(0, 2, 1, 3)
    hv = (kv64 @ Wv.T.astype(np.float64)).reshape(B, L, NH, D).transpose(0, 2, 1, 3)
    s = np.einsum("bhqd,bhkd->bhqk", hq, hk) / math.sqrt(D)
    mask = attention_mask[:, None, :, :]
    s = np.where(mask, s, -np.inf)
    s = s - s.max(axis=-1, keepdims=True)
    e = np.exp(s)
    p = e / np.maximum(e.sum(axis=-1, keepdims=True), 1e-300)
    p = np.where(mask, p, 0.0)
    o = np.einsum("bhqk,bhkd->bhqd", p, hv)
    o = o.transpose(0, 2, 1, 3).reshape(B, L, NH * D)
    return (o @ Wo.T.astype(np.float64)).astype(np.float32)


def kernel(query, key_value, attention_mask, Wq, Wk, Wv, Wo):
    query = np.asarray(query)
    key_value = np.asarray(key_value)
    attention_mask = np.asarray(attention_mask)
    Wq, Wk, Wv, Wo = (np.asarray(a) for a in (Wq, Wk, Wv, Wo))

    if not attention_mask.all():
        return _numpy_fallback(query, key_value, attention_mask, Wq, Wk, Wv, Wo)

    from concourse.bass_utils import run_bass_kernel_spmd

    nc = _get_nc()
    in_maps = make_in_maps(query, key_value, Wq, Wk, Wv, Wo)
    res = run_bass_kernel_spmd(nc, in_maps, list(range(NC_))).results
    out = np.zeros((B, L, DOUT), np.float32)
    for core in range(NC_):
        b = core // (NC_ // B)
        out[b] += res[core]["out"].astype(np.float32)
    return out


# revision 30
# speedup vs baseline: 1.4084x; 1.0102x over previous
"""Multi-head attention (B=2, L=2048, D=2048, 16 heads x 128) on 8 trn2 cores.

Sharding: tensor-parallel over heads (4 groups of 4 heads) x data-parallel
over batch (2) -> 8 cores.  Each core computes, for its (batch b, group g):
    hq = q_b @ Wq_g.T, hk = kv_b @ Wk_g.T, hv = kv_b @ Wv_g.T   (4 heads)
    per head: P = softmax(hq hk^T / sqrt(128)), o = P hv
    partial_out = concat_heads(o) @ Wo[:, g].T        [2048, 2048]
Host sums the 4 per-group partials for each batch.

All matmuls run in bf16 (fp32 PSUM accumulation).  bf16 stationary weight
loads are 1 cycle/row on the PE (vs ~4 for float32r), which removes most of
the LDWEIGHTS overhead that dominated the f32r version; bf16 also halves
DMA traffic.  End-to-end numeric error ~5e-3 (tolerance 2e-2).

The softmax denominator is computed off the PE: a bf16 pairwise tree of
DVE adds folds the 16 exp tiles into one [128, 512] partial, and a single
ones-matmul on the PE does the final cross-partition reduction.

Device layout (per core):
  xblk     [128, 16c x 512]   column block of qT/kvT, streamed per n
  hqT/hkT  [128 d, 4h x 2048 seq]  (d on partitions)
  hv       [128 k, 16 kt x 512(=4h x 128 d)]
  scores^T [128 k-tile, 2x512 q] in PSUM -> exp on ACT -> SBUF bf16
  AV:      o^T[128 d, 512 q] += hv_kt.T @ exp_kt  (PSUM accumulate)
  denom:   DVE bf16 tree -> d128; ones-matmul -> ps_d; recip+mul on DVE
  Wo:      out[128 q, 2048 dout] += o_chunk.T @ woT_chunk, per q-tile
"""
import math
import sys

for _p in ("/opt/trn_rl_repo", "/root/.axon_site/_ro/trn_rl_repo"):
    if _p not in sys.path:
        sys.path.append(_p)

import numpy as np
import ml_dtypes

B = 2
L = 2048           # LQ == LK
DIN = 2048
NH = 16            # total heads
HL = 4             # heads per core
D = 128            # head dim
HD = HL * D        # 512, head-group width
DOUT = 2048
NC_ = 8            # cores
NCH = DIN // 128   # 16 contraction chunks
NQ = 4             # q blocks of 512
QB = 512
NKT = L // 128     # 16 key tiles

_CACHE = {}


def _build_nc():
    import concourse.bacc as bacc
    import concourse.mybir as mybir
    import concourse.tile as tile

    BF16 = mybir.dt.bfloat16
    F32 = mybir.dt.float32

    nc = bacc.Bacc("TRN2", target_bir_lowering=False, debug=False)
    qT = nc.dram_tensor("qT", [DIN, L], BF16, kind="ExternalInput").ap()
    kvT = nc.dram_tensor("kvT", [DIN, L], BF16, kind="ExternalInput").ap()
    wqT = nc.dram_tensor("wqT", [DIN, HD], BF16, kind="ExternalInput").ap()
    wkT = nc.dram_tensor("wkT", [DIN, HD], BF16, kind="ExternalInput").ap()
    wvT = nc.dram_tensor("wvT", [DIN, HD], BF16, kind="ExternalInput").ap()
    woT = nc.dram_tensor("woT", [HD, DOUT], BF16, kind="ExternalInput").ap()
    allones = nc.dram_tensor("allones", [128, 128], BF16, kind="ExternalInput").ap()
    out = nc.dram_tensor("out", [L, DOUT], BF16, kind="ExternalOutput").ap()

    EXP = mybir.ActivationFunctionType.Exp
    COPY = mybir.ActivationFunctionType.Copy

    with tile.TileContext(nc) as tc:
        with (
            nc.allow_low_precision(reason="bf16 matmuls, ~5e-3 relmax vs 2e-2 tol"),
            tc.tile_pool(name="persist", bufs=1) as pp,
            tc.tile_pool(name="psum", bufs=2, space="PSUM") as psp,
        ):
            hq_sb = pp.tile([128, HL * L], BF16, tag="hq")
            hk_sb = pp.tile([128, HL * L], BF16, tag="hk")
            hv_sb = pp.tile([128, NKT * HD], BF16, tag="hv")
            o_sb = pp.tile([128, HL * L], BF16, tag="o")
            wo_sb = pp.tile([128, HL * DOUT], BF16, tag="wo")
            ones_sb = pp.tile([128, 128], BF16, tag="ones")
            nc.gpsimd.dma_start(out=ones_sb[:], in_=allones)

            # ---------------- projections ----------------
            with tc.tile_pool(name="proj", bufs=1) as jp:
                # wk and the first kv block gate the first matmul: stream both
                # in interleaved 4-chunk pieces so c=0 can start early.
                PIECES = [(0, 2), (2, 6), (6, 11), (11, 16)]

                def load_w_piece(w_sb, w_dram, piece):
                    c0, c1 = PIECES[piece]
                    nc.gpsimd.dma_start(
                        out=w_sb[:, c0 * HD : c1 * HD].rearrange(
                            "p (c m) -> p c m", m=HD
                        ),
                        in_=w_dram[c0 * 128 : c1 * 128, :].rearrange(
                            "(c p) m -> p c m", p=128
                        ),
                    )

                w_sbs = {
                    nm: jp.tile([128, NCH * HD], BF16, tag=f"w_{nm}", name=nm)
                    for nm in ("wk", "wv", "wq")
                }
                xblk0 = jp.tile([128, NCH * QB], BF16, tag="xblk", bufs=3, name="xblk")

                def load_x_piece(xblk, x_dram, n, piece):
                    c0, c1 = PIECES[piece]
                    nc.sync.dma_start(
                        out=xblk[:, c0 * QB : c1 * QB].rearrange(
                            "p (c q) -> p c q", q=QB
                        ),
                        in_=x_dram[c0 * 128 : c1 * 128, n * QB : (n + 1) * QB].rearrange(
                            "(c p) q -> p c q", p=128
                        ),
                    )

                xblk1 = jp.tile([128, NCH * QB], BF16, tag="xblk", bufs=3, name="xblk")
                for piece in range(4):
                    load_w_piece(w_sbs["wk"], wkT, piece)
                    load_x_piece(xblk0, kvT, 0, piece)
                # wv (needed by V-n0) and kv block 1 (needed by K-n1) stream
                # together so both have their early chunks in time.
                for piece in range(4):
                    load_w_piece(w_sbs["wv"], wvT, piece)
                    load_x_piece(xblk1, kvT, 1, piece)
                # wq/wo are not needed until the Q pass / attention: loaded
                # after the KV-pass emission so their packets don't compete
                # with the kv column blocks.

                def proj_block(x_sb, w_sb, accs, is_v):
                    for c in range(NCH):
                        for j in range(4):
                            if is_v:
                                nc.tensor.matmul(
                                    accs[j][:],
                                    x_sb[:, c * HD + j * 128 : c * HD + (j + 1) * 128],
                                    w_sb[:, c * HD : (c + 1) * HD],
                                    start=(c == 0),
                                    stop=(c == NCH - 1),
                                )
                            else:
                                nc.tensor.matmul(
                                    accs[j][:],
                                    w_sb[:, c * HD + j * 128 : c * HD + (j + 1) * 128],
                                    x_sb[:, c * HD : (c + 1) * HD],
                                    start=(c == 0),
                                    stop=(c == NCH - 1),
                                )

                # PSUM budget is 8 banks: tag "ps" [128,1024] bufs=2 (4 banks)
                # + "po" [128,512] bufs=2 (2) + "pd"/"wo" [128,512] bufs=1
                # (1 each).  Projections need 4 accumulators double-buffered
                # across n: two in "ps", one in "po", and the fourth
                # alternating between "pd" and "wo" by block parity.
                def new_accs(parity):
                    a = psp.tile([128, 2 * QB], F32, tag="ps", name="accA")
                    b = psp.tile([128, QB], F32, tag="po", name="accB")
                    c = psp.tile(
                        [128, QB], F32, tag=("pd" if parity == 0 else "wo"),
                        bufs=1, name="accC",
                    )
                    return [a[:, 0:QB], a[:, QB : 2 * QB], b[:], c[:]]

                def load_xblk(x_dram, n):
                    xblk = jp.tile(
                        [128, NCH * QB], BF16, tag="xblk", bufs=3, name="xblk"
                    )
                    nc.sync.dma_start(
                        out=xblk.rearrange("p (c q) -> p c q", q=QB),
                        in_=x_dram[:, n * QB : (n + 1) * QB].rearrange(
                            "(c p) q -> p c q", p=128
                        ),
                    )
                    return xblk

                preloaded = {0: xblk0, 1: xblk1}
                for n in range(NQ):
                    xblk = preloaded[n] if n in preloaded else load_xblk(kvT, n)
                    # K projection: hk^T[d, seq], stationary = wk chunk
                    accs = new_accs(0)
                    proj_block(xblk, w_sbs["wk"], accs, is_v=False)
                    for j in range(4):
                        nc.scalar.activation(
                            hk_sb[:, j * L + n * QB : j * L + (n + 1) * QB],
                            accs[j][:],
                            COPY,
                        )
                    # V projection: hv[k, 4h*128d], stationary = kv chunk cols
                    accs = new_accs(1)
                    proj_block(xblk, w_sbs["wv"], accs, is_v=True)
                    for j in range(4):
                        nc.scalar.activation(
                            hv_sb[:, (n * 4 + j) * HD : (n * 4 + j + 1) * HD],
                            accs[j][:],
                            COPY,
                        )
                for piece in range(4):
                    load_w_piece(w_sbs["wq"], wqT, piece)
                for h in range(HL):
                    nc.gpsimd.dma_start(
                        out=wo_sb[:, h * DOUT : (h + 1) * DOUT],
                        in_=woT[h * 128 : (h + 1) * 128, :],
                    )
                for n in range(NQ):
                    xblk = load_xblk(qT, n)
                    accs = new_accs(n % 2)
                    proj_block(xblk, w_sbs["wq"], accs, is_v=False)
                    for j in range(4):
                        # last pass drains split ACT/DVE so psum slots free
                        # quickly and ACT is soon clear for the first exps
                        if n == NQ - 1 and j % 2 == 0:
                            nc.vector.tensor_copy(
                                out=hq_sb[:, j * L + n * QB : j * L + (n + 1) * QB],
                                in_=accs[j][:],
                            )
                        else:
                            nc.scalar.activation(
                                hq_sb[:, j * L + n * QB : j * L + (n + 1) * QB],
                                accs[j][:],
                                COPY,
                            )

            # ---------------- attention ----------------
            with tc.tile_pool(name="attn", bufs=1) as ap:
                def flush(st):
                    # deferred normalization of the previous (n, h) iteration
                    h_, n_, ps_o_, ps_d_ = st
                    recip = ap.tile([128, QB], F32, tag="recip", bufs=2, name="recip")
                    nc.vector.reciprocal_approx_fast(out=recip[:], in_=ps_d_)
                    nc.vector.tensor_mul(
                        out=o_sb[:, h_ * L + n_ * QB : h_ * L + (n_ + 1) * QB],
                        in0=ps_o_,
                        in1=recip[:],
                    )

                # Flat software pipeline over the 16 (n, h) units: AV lags
                # scores by one kt-pair within a unit; the last AV pair, the
                # ones-matmul (partition reduction) and the normalization of
                # unit u are deferred into unit u+1 so the PE never waits on
                # ACT/DVE at a unit boundary.  Attention is ACT-bound (8 exps
                # x ~1.1us/unit vs ~7.1us of PE work), so Wo output-projection
                # groups are interleaved into the spare PE slots as soon as
                # their q-block is normalized; only block 3's Wo remains as a
                # short tail.
                units = [(n, h) for n in range(NQ) for h in range(HL)]
                state = {}  # u -> dict with ps_o, ps_d, exp_half, d128, (n, h)
                prev_flush = None
                wo_queue = []  # ready (qt, m) output groups
                wo_stage_eng = [0]  # alternate DVE for interleaved drains

                def emit_av(u, pair):
                    st = state[u]
                    n_, h_ = st["nh"]
                    for t in range(2):
                        kt = 2 * pair + t
                        e_sl = st["exp_half"][kt // 8][
                            :, (kt % 8) * QB : (kt % 8 + 1) * QB
                        ]
                        nc.tensor.matmul(
                            st["ps_o"][:],
                            hv_sb[:, kt * HD + h_ * 128 : kt * HD + (h_ + 1) * 128],
                            e_sl,
                            start=(kt == 0),
                            stop=(kt == NKT - 1),
                        )

                def emit_wo_group(qt, m, tag, on_act):
                    acc = psp.tile(
                        [128, QB], F32, tag=tag,
                        bufs=(1 if tag in ("wo", "pd") else 2),
                        name="woacc",
                    )
                    for h_ in range(HL):
                        nc.tensor.matmul(
                            acc[:],
                            o_sb[:, h_ * L + qt * 128 : h_ * L + (qt + 1) * 128],
                            wo_sb[:, h_ * DOUT + m * QB : h_ * DOUT + (m + 1) * QB],
                            start=(h_ == 0),
                            stop=(h_ == HL - 1),
                        )
                    stage = ap.tile([128, QB], BF16, tag="wstage", bufs=4, name="ws")
                    if on_act:
                        nc.scalar.activation(stage[:], acc[:], COPY)
                    else:
                        nc.vector.tensor_copy(out=stage[:], in_=acc[:])
                    nc.sync.dma_start(
                        out=out[qt * 128 : (qt + 1) * 128, m * QB : (m + 1) * QB],
                        in_=stage[:],
                    )

                for u, (n, h) in enumerate(units):
                    hq_sl = hq_sb[:, h * L + n * QB : h * L + (n + 1) * QB]
                    st = {
                        "nh": (n, h),
                        "ps_o": psp.tile([128, QB], F32, tag="po", name="ps_o"),
                        "ps_d": psp.tile([128, QB], F32, tag="pd", bufs=1, name="ps_d"),
                        "exp_half": [None, None],
                    }
                    state[u] = st
                    wo_emitted = 0
                    tl = {}  # denominator tree: level -> pending tiles
                    for p in range(8):
                        half = p // 4
                        if p % 4 == 0:
                            st["exp_half"][half] = ap.tile(
                                [128, 8 * QB], BF16, tag="exp", bufs=3, name="exp"
                            )
                        off = (p % 4) * 2 * QB
                        ps_s = psp.tile([128, 2 * QB], F32, tag="ps", name="ps_s")
                        for t in range(2):
                            kt = 2 * p + t
                            nc.tensor.matmul(
                                ps_s[:, t * QB : (t + 1) * QB],
                                hk_sb[:, h * L + kt * 128 : h * L + (kt + 1) * 128],
                                hq_sl,
                                start=True,
                                stop=True,
                            )
                        nc.scalar.activation(
                            st["exp_half"][half][:, off : off + 2 * QB], ps_s[:], EXP
                        )
                        # level-0 tree add over this exp pair, fold-up when ready
                        t0 = ap.tile([128, QB], BF16, tag="t0", bufs=2, name="t0")
                        nc.vector.tensor_add(
                            out=t0[:],
                            in0=st["exp_half"][half][:, off : off + QB],
                            in1=st["exp_half"][half][:, off + QB : off + 2 * QB],
                        )
                        tl.setdefault(0, []).append(t0)
                        lv = 0
                        while len(tl.get(lv, [])) == 2:
                            a, b = tl.pop(lv)
                            nxt = ap.tile(
                                [128, QB], BF16, tag=f"t{lv+1}", bufs=2,
                                name=f"t{lv+1}",
                            )
                            nc.vector.tensor_add(out=nxt[:], in0=a[:], in1=b[:])
                            tl.setdefault(lv + 1, []).append(nxt)
                            lv += 1
                        # deferred work from the previous unit / this unit.
                        # AV lags scores by TWO kt-pairs so it never races the
                        # exp pipeline; the last two pairs land in unit u+1.
                        if p == 0 and u > 0:
                            emit_av(u - 1, 6)
                        if p == 1 and u > 0:
                            emit_av(u - 1, 7)
                        if p >= 2:
                            emit_av(u, p - 2)
                        if p == 2 and u > 0:
                            pst = state[u - 1]
                            nc.tensor.matmul(
                                pst["ps_d"][:], ones_sb[:], pst["d128"][:],
                                start=True, stop=True,
                            )
                            n_, h_ = pst["nh"]
                            prev_flush = (h_, n_, pst["ps_o"], pst["ps_d"])
                        if p == 4 and prev_flush is not None:
                            fh, fn, fo, fd = prev_flush
                            flush(prev_flush)
                            prev_flush = None
                            if fh == HL - 1:
                                # block fn fully normalized: queue its Wo groups
                                wo_queue.extend(
                                    (fn * 4 + qq, m)
                                    for qq in range(4)
                                    for m in range(4)
                                )
                        if p >= 2 and p != 4 and wo_queue and wo_emitted < 6:
                            qt_, m_ = wo_queue.pop(0)
                            # drains alternate DVE/ACT to keep both under PE
                            emit_wo_group(qt_, m_, "wo", on_act=(wo_emitted % 2 == 1))
                            wo_emitted += 1
                    st["d128"] = tl[3][0]
                    if u > 0:
                        state.pop(u - 1)
                # drain the pipeline.  Block 3's Wo tail is the only PE work
                # left, and its h=3 inputs depend on the final flush chain —
                # so the first five tail groups' h=0..2 matmuls are emitted
                # FIRST (their inputs are already flushed) to keep the PE busy
                # while ACT finishes the last exps and DVE runs the flushes.
                last = len(units) - 1
                st = state[last]
                wo_queue.extend((3 * 4 + qq, m) for qq in range(4) for m in range(4))
                tail_tags = ["wo", "po", "pd"]

                def wo_mm(acc, qt_, m_, h_):
                    nc.tensor.matmul(
                        acc,
                        o_sb[:, h_ * L + qt_ * 128 : h_ * L + (qt_ + 1) * 128],
                        wo_sb[:, h_ * DOUT + m_ * QB : h_ * DOUT + (m_ + 1) * QB],
                        start=(h_ == 0),
                        stop=(h_ == HL - 1),
                    )

                # NOTE: tag "pd" is NOT usable here — its single bank still
                # belongs to ps_d(last), which the ones-matmul below writes;
                # using it would deadlock PE program order against the flush.
                head = wo_queue[:4]
                ps_pair = psp.tile([128, 2 * QB], F32, tag="ps", name="ps_tail")
                head_accs = [
                    psp.tile([128, QB], F32, tag="wo", bufs=1, name="woacc")[:],
                    psp.tile([128, QB], F32, tag="po", name="woacc")[:],
                    ps_pair[:, 0:QB],
                    ps_pair[:, QB : 2 * QB],
                ]
                for i, (qt_, m_) in enumerate(head):
                    for h_ in range(HL - 1):
                        wo_mm(head_accs[i], qt_, m_, h_)
                # last unit's deferred AV pairs, reduction, final flushes
                emit_av(last, 6)
                emit_av(last, 7)
                nc.tensor.matmul(
                    st["ps_d"][:], ones_sb[:], st["d128"][:], start=True, stop=True
                )
                if prev_flush is not None:  # unit last-1, ones'd at p2 of last
                    flush(prev_flush)
                n_, h_ = st["nh"]
                flush((h_, n_, st["ps_o"], st["ps_d"]))
                for i, (qt_, m_) in enumerate(head):
                    wo_mm(head_accs[i], qt_, m_, HL - 1)
                    stage = ap.tile([128, QB], BF16, tag="wstage", bufs=4, name="ws")
                    nc.scalar.activation(stage[:], head_accs[i], COPY)
                    nc.sync.dma_start(
                        out=out[qt_ * 128 : (qt_ + 1) * 128, m_ * QB : (m_ + 1) * QB],
                        in_=stage[:],
                    )
                for i, (qt_, m_) in enumerate(wo_queue[4:]):
                    emit_wo_group(qt_, m_, tail_tags[i % 3], on_act=(i % 2 == 0))
    nc.compile()
    return nc


def _get_nc():
    if "nc" not in _CACHE:
        _CACHE["nc"] = _build_nc()
    return _CACHE["nc"]


def make_in_maps(query, key_value, Wq, Wk, Wv, Wo):
    scale = 1.0 / math.sqrt(D)
    bf = ml_dtypes.bfloat16
    allones = np.ones((128, 128), bf)
    in_maps = []
    qT = [np.ascontiguousarray(query[b].T.astype(bf)) for b in range(B)]
    kvT = [np.ascontiguousarray(key_value[b].T.astype(bf)) for b in range(B)]
    for core in range(NC_):
        b, g = divmod(core, NC_ // B)
        sl = slice(g * HD, (g + 1) * HD)
        in_maps.append(
            {
                "qT": qT[b],
                "kvT": kvT[b],
                "wqT": np.ascontiguousarray((Wq[sl, :] * scale).T.astype(bf)),
                "wkT": np.ascontiguousarray(Wk[sl, :].T.astype(bf)),
                "wvT": np.ascontiguousarray(Wv[sl, :].T.astype(bf)),
                "woT": np.ascontiguousarray(Wo[:, sl].T.astype(bf)),
                "allones": allones,
            }
        )
    return in_maps


def _numpy_fallback(query, key_value, attention_mask, Wq, Wk, Wv, Wo):
    # Only reached if the mask is not all-ones (never per the problem spec).
    q64, kv64 = query.astype(np.float64), key_value.astype(np.float64)
    hq = (q64 @ Wq.T.astype(np.float64)).reshape(B, L, NH, D).transpose(0, 2, 1, 3)
    hk = (kv64 @ Wk.T.astype(np.float64)).reshape(B, L, NH, D).transpose(0, 2, 1, 3)
    hv = (kv64 @ Wv.T.astype(np.float64)).reshape(B, L, NH, D).transpose(0, 2, 1, 3)
    s = np.einsum("bhqd,bhkd->bhqk", hq, hk) / math.sqrt(D)
    mask = attention_mask[:, None, :, :]
    s = np.where(mask, s, -np.inf)
    s = s - s.max(axis=-1, keepdims=True)
    e = np.exp(s)
    p = e / np.maximum(e.sum(axis=-1, keepdims=True), 1e-300)
    p = np.where(mask, p, 0.0)
    o = np.einsum("bhqk,bhkd->bhqd", p, hv)
    o = o.transpose(0, 2, 1, 3).reshape(B, L, NH * D)
    return (o @ Wo.T.astype(np.float64)).astype(np.float32)


def kernel(query, key_value, attention_mask, Wq, Wk, Wv, Wo):
    query = np.asarray(query)
    key_value = np.asarray(key_value)
    attention_mask = np.asarray(attention_mask)
    Wq, Wk, Wv, Wo = (np.asarray(a) for a in (Wq, Wk, Wv, Wo))

    if not attention_mask.all():
        return _numpy_fallback(query, key_value, attention_mask, Wq, Wk, Wv, Wo)

    from concourse.bass_utils import run_bass_kernel_spmd

    nc = _get_nc()
    in_maps = make_in_maps(query, key_value, Wq, Wk, Wv, Wo)
    res = run_bass_kernel_spmd(nc, in_maps, list(range(NC_))).results
    out = np.zeros((B, L, DOUT), np.float32)
    for core in range(NC_):
        b = core // (NC_ // B)
        out[b] += res[core]["out"].astype(np.float32)
    return out


# revision 37
# speedup vs baseline: 1.4434x; 1.0249x over previous
"""Multi-head attention (B=2, L=2048, D=2048, 16 heads x 128) on 8 trn2 cores.

Sharding: tensor-parallel over heads (4 groups of 4 heads) x data-parallel
over batch (2) -> 8 cores.  Each core computes, for its (batch b, group g):
    hq = q_b @ Wq_g.T, hk = kv_b @ Wk_g.T, hv = kv_b @ Wv_g.T   (4 heads)
    per head: P = softmax(hq hk^T / sqrt(128)), o = P hv
    partial_out = concat_heads(o) @ Wo[:, g].T        [2048, 2048]
Host sums the 4 per-group partials for each batch.

All matmuls run in bf16 (fp32 PSUM accumulation).  bf16 stationary weight
loads are 1 cycle/row on the PE (vs ~4 for float32r), which removes most of
the LDWEIGHTS overhead that dominated the f32r version; bf16 also halves
DMA traffic.  End-to-end numeric error ~5e-3 (tolerance 2e-2).

The softmax denominator is computed off the PE: a bf16 pairwise tree of
DVE adds folds the 16 exp tiles into one [128, 512] partial, and a single
ones-matmul on the PE does the final cross-partition reduction.

Device layout (per core):
  xblk     [128, 16c x 512]   column block of qT/kvT, streamed per n
  hqT/hkT  [128 d, 4h x 2048 seq]  (d on partitions)
  hv       [128 k, 16 kt x 512(=4h x 128 d)]
  scores^T [128 k-tile, 2x512 q] in PSUM -> exp on ACT -> SBUF bf16
  AV:      o^T[128 d, 512 q] += hv_kt.T @ exp_kt  (PSUM accumulate)
  denom:   DVE bf16 tree -> d128; ones-matmul -> ps_d; recip+mul on DVE
  Wo:      out[128 q, 2048 dout] += o_chunk.T @ woT_chunk, per q-tile
"""
import math
import sys

for _p in ("/opt/trn_rl_repo", "/root/.axon_site/_ro/trn_rl_repo"):
    if _p not in sys.path:
        sys.path.append(_p)

import numpy as np
import ml_dtypes

B = 2
L = 2048           # LQ == LK
DIN = 2048
NH = 16            # total heads
HL = 4             # heads per core
D = 128            # head dim
HD = HL * D        # 512, head-group width
DOUT = 2048
NC_ = 8            # cores
NCH = DIN // 128   # 16 contraction chunks
NQ = 4             # q blocks of 512
QB = 512
NKT = L // 128     # 16 key tiles

_CACHE = {}


def _build_nc():
    import concourse.bacc as bacc
    import concourse.mybir as mybir
    import concourse.tile as tile

    BF16 = mybir.dt.bfloat16
    F32 = mybir.dt.float32

    nc = bacc.Bacc("TRN2", target_bir_lowering=False, debug=False)
    qT = nc.dram_tensor("qT", [DIN, L], BF16, kind="ExternalInput").ap()
    kvT = nc.dram_tensor("kvT", [DIN, L], BF16, kind="ExternalInput").ap()
    wqT = nc.dram_tensor("wqT", [DIN, HD], BF16, kind="ExternalInput").ap()
    wkT = nc.dram_tensor("wkT", [DIN, HD], BF16, kind="ExternalInput").ap()
    wvT = nc.dram_tensor("wvT", [DIN, HD], BF16, kind="ExternalInput").ap()
    woT = nc.dram_tensor("woT", [HD, DOUT], BF16, kind="ExternalInput").ap()
    allones = nc.dram_tensor("allones", [128, 128], BF16, kind="ExternalInput").ap()
    out = nc.dram_tensor("out", [L, DOUT], BF16, kind="ExternalOutput").ap()

    EXP = mybir.ActivationFunctionType.Exp
    COPY = mybir.ActivationFunctionType.Copy

    with tile.TileContext(nc) as tc:
        with (
            nc.allow_low_precision(reason="bf16 matmuls, ~5e-3 relmax vs 2e-2 tol"),
            tc.tile_pool(name="persist", bufs=1) as pp,
            tc.tile_pool(name="psum", bufs=2, space="PSUM") as psp,
        ):
            hq_sb = pp.tile([128, HL * L], BF16, tag="hq")
            hk_sb = pp.tile([128, HL * L], BF16, tag="hk")
            hv_sb = pp.tile([128, NKT * HD], BF16, tag="hv")
            o_sb = pp.tile([128, HL * L], BF16, tag="o")
            wo_sb = pp.tile([128, HL * DOUT], BF16, tag="wo")
            ones_sb = pp.tile([128, 128], BF16, tag="ones")
            # wq and the last q block persist into the attention phase: the
            # Q-projection of block 3 is interleaved into the early attention
            # units (which are otherwise ACT-bound), since no attention unit
            # reads hq block 3 before unit 12.
            wq_sb = pp.tile([128, NCH * HD], BF16, tag="wq")
            xblk_q3 = pp.tile([128, NCH * QB], BF16, tag="xq3")
            nc.gpsimd.dma_start(out=ones_sb[:], in_=allones)

            # ---------------- projections ----------------
            with tc.tile_pool(name="proj", bufs=1) as jp:
                # wk and the first kv block gate the first matmul: stream both
                # in interleaved 4-chunk pieces so c=0 can start early.
                PIECES = [(0, 2), (2, 6), (6, 11), (11, 16)]

                def load_w_piece(w_sb, w_dram, piece):
                    c0, c1 = PIECES[piece]
                    nc.gpsimd.dma_start(
                        out=w_sb[:, c0 * HD : c1 * HD].rearrange(
                            "p (c m) -> p c m", m=HD
                        ),
                        in_=w_dram[c0 * 128 : c1 * 128, :].rearrange(
                            "(c p) m -> p c m", p=128
                        ),
                    )

                w_sbs = {
                    nm: jp.tile([128, NCH * HD], BF16, tag=f"w_{nm}", name=nm)
                    for nm in ("wk", "wv")
                }
                w_sbs["wq"] = wq_sb
                xblk0 = jp.tile([128, NCH * QB], BF16, tag="xblk", bufs=3, name="xblk")

                def load_x_piece(xblk, x_dram, n, piece):
                    c0, c1 = PIECES[piece]
                    nc.sync.dma_start(
                        out=xblk[:, c0 * QB : c1 * QB].rearrange(
                            "p (c q) -> p c q", q=QB
                        ),
                        in_=x_dram[c0 * 128 : c1 * 128, n * QB : (n + 1) * QB].rearrange(
                            "(c p) q -> p c q", p=128
                        ),
                    )

                xblk1 = jp.tile([128, NCH * QB], BF16, tag="xblk", bufs=3, name="xblk")
                for piece in range(4):
                    load_w_piece(w_sbs["wk"], wkT, piece)
                    load_x_piece(xblk0, kvT, 0, piece)
                # wv (needed by V-n0) and kv block 1 (needed by K-n1) stream
                # together so both have their early chunks in time.
                for piece in range(4):
                    load_w_piece(w_sbs["wv"], wvT, piece)
                    load_x_piece(xblk1, kvT, 1, piece)
                # wq/wo are not needed until the Q pass / attention: loaded
                # after the KV-pass emission so their packets don't compete
                # with the kv column blocks.

                def proj_block(x_sb, w_sb, accs, is_v):
                    for c in range(NCH):
                        for j in range(4):
                            if is_v:
                                nc.tensor.matmul(
                                    accs[j][:],
                                    x_sb[:, c * HD + j * 128 : c * HD + (j + 1) * 128],
                                    w_sb[:, c * HD : (c + 1) * HD],
                                    start=(c == 0),
                                    stop=(c == NCH - 1),
                                )
                            else:
                                nc.tensor.matmul(
                                    accs[j][:],
                                    w_sb[:, c * HD + j * 128 : c * HD + (j + 1) * 128],
                                    x_sb[:, c * HD : (c + 1) * HD],
                                    start=(c == 0),
                                    stop=(c == NCH - 1),
                                )

                # PSUM budget is 8 banks: tag "ps" [128,1024] bufs=2 (4 banks)
                # + "po" [128,512] bufs=2 (2) + "pd"/"wo" [128,512] bufs=1
                # (1 each).  Projections need 4 accumulators double-buffered
                # across n: two in "ps", one in "po", and the fourth
                # alternating between "pd" and "wo" by block parity.
                def new_accs(parity):
                    a = psp.tile([128, 2 * QB], F32, tag="ps", name="accA")
                    b = psp.tile([128, QB], F32, tag="po", name="accB")
                    c = psp.tile(
                        [128, QB], F32, tag=("pd" if parity == 0 else "wo"),
                        bufs=1, name="accC",
                    )
                    return [a[:, 0:QB], a[:, QB : 2 * QB], b[:], c[:]]

                def load_xblk(x_dram, n):
                    xblk = jp.tile(
                        [128, NCH * QB], BF16, tag="xblk", bufs=3, name="xblk"
                    )
                    nc.sync.dma_start(
                        out=xblk.rearrange("p (c q) -> p c q", q=QB),
                        in_=x_dram[:, n * QB : (n + 1) * QB].rearrange(
                            "(c p) q -> p c q", p=128
                        ),
                    )
                    return xblk

                preloaded = {0: xblk0, 1: xblk1}
                for n in range(NQ):
                    xblk = preloaded[n] if n in preloaded else load_xblk(kvT, n)
                    # K projection: hk^T[d, seq], stationary = wk chunk
                    accs = new_accs(0)
                    proj_block(xblk, w_sbs["wk"], accs, is_v=False)
                    for j in range(4):
                        nc.scalar.activation(
                            hk_sb[:, j * L + n * QB : j * L + (n + 1) * QB],
                            accs[j][:],
                            COPY,
                        )
                    # V projection: hv[k, 4h*128d], stationary = kv chunk cols
                    accs = new_accs(1)
                    proj_block(xblk, w_sbs["wv"], accs, is_v=True)
                    for j in range(4):
                        nc.scalar.activation(
                            hv_sb[:, (n * 4 + j) * HD : (n * 4 + j + 1) * HD],
                            accs[j][:],
                            COPY,
                        )
                for piece in range(4):
                    load_w_piece(w_sbs["wq"], wqT, piece)
                for h in range(HL):
                    nc.gpsimd.dma_start(
                        out=wo_sb[:, h * DOUT : (h + 1) * DOUT],
                        in_=woT[h * 128 : (h + 1) * 128, :],
                    )
                for n in range(NQ - 1):
                    if n == 1:
                        # block 3's q columns: needed only by the attention
                        # phase, so issued behind the first q block
                        nc.sync.dma_start(
                            out=xblk_q3.rearrange("p (c q) -> p c q", q=QB),
                            in_=qT[:, 3 * QB : 4 * QB].rearrange(
                                "(c p) q -> p c q", p=128
                            ),
                        )
                    xblk = load_xblk(qT, n)
                    accs = new_accs(n % 2)
                    proj_block(xblk, w_sbs["wq"], accs, is_v=False)
                    for j in range(4):
                        # last in-loop pass drains split ACT/DVE so psum slots
                        # free quickly and ACT is soon clear for the first exps
                        if n == NQ - 2 and j % 2 == 0:
                            nc.vector.tensor_copy(
                                out=hq_sb[:, j * L + n * QB : j * L + (n + 1) * QB],
                                in_=accs[j][:],
                            )
                        else:
                            nc.scalar.activation(
                                hq_sb[:, j * L + n * QB : j * L + (n + 1) * QB],
                                accs[j][:],
                                COPY,
                            )

            # ---------------- attention ----------------
            with tc.tile_pool(name="attn", bufs=1) as ap:
                def flush(st):
                    # deferred normalization of the previous (n, h) iteration
                    h_, n_, ps_o_, ps_d_ = st
                    recip = ap.tile([128, QB], F32, tag="recip", bufs=2, name="recip")
                    nc.vector.reciprocal_approx_fast(out=recip[:], in_=ps_d_)
                    nc.vector.tensor_mul(
                        out=o_sb[:, h_ * L + n_ * QB : h_ * L + (n_ + 1) * QB],
                        in0=ps_o_,
                        in1=recip[:],
                    )

                # Flat software pipeline over the 16 (n, h) units: AV lags
                # scores by one kt-pair within a unit; the last AV pair, the
                # ones-matmul (partition reduction) and the normalization of
                # unit u are deferred into unit u+1 so the PE never waits on
                # ACT/DVE at a unit boundary.  Attention is ACT-bound (8 exps
                # x ~1.1us/unit vs ~7.1us of PE work), so Wo output-projection
                # groups are interleaved into the spare PE slots as soon as
                # their q-block is normalized; only block 3's Wo remains as a
                # short tail.
                units = [(n, h) for n in range(NQ) for h in range(HL)]
                state = {}  # u -> dict with ps_o, ps_d, exp_half, d128, (n, h)
                prev_flush = None
                wo_queue = []  # ready (qt, m) output groups
                # deferred Q-projection of block 3 (PE filler for the
                # ACT-bound units 0..3, which never read hq block 3)
                q3_ops = [(j, c) for j in range(4) for c in range(NCH)]
                q3_acc = [None]

                def emit_q3(count):
                    for _ in range(count):
                        if not q3_ops:
                            return
                        j, c = q3_ops.pop(0)
                        if c == 0:
                            q3_acc[0] = psp.tile(
                                [128, QB], F32, tag="wo", bufs=1, name="q3acc"
                            )
                        nc.tensor.matmul(
                            q3_acc[0][:],
                            wq_sb[:, c * HD + j * 128 : c * HD + (j + 1) * 128],
                            xblk_q3[:, c * QB : (c + 1) * QB],
                            start=(c == 0),
                            stop=(c == NCH - 1),
                        )
                        if c == NCH - 1:
                            nc.vector.tensor_copy(
                                out=hq_sb[:, j * L + 3 * QB : j * L + 4 * QB],
                                in_=q3_acc[0][:],
                            )

                def emit_av(u, pair):
                    st = state[u]
                    n_, h_ = st["nh"]
                    for t in range(2):
                        kt = 2 * pair + t
                        e_sl = st["exp_half"][kt // 8][
                            :, (kt % 8) * QB : (kt % 8 + 1) * QB
                        ]
                        nc.tensor.matmul(
                            st["ps_o"][:],
                            hv_sb[:, kt * HD + h_ * 128 : kt * HD + (h_ + 1) * 128],
                            e_sl,
                            start=(kt == 0),
                            stop=(kt == NKT - 1),
                        )

                def emit_wo_group(qt, m, tag, on_act):
                    acc = psp.tile(
                        [128, QB], F32, tag=tag,
                        bufs=(1 if tag in ("wo", "pd") else 2),
                        name="woacc",
                    )
                    for h_ in range(HL):
                        nc.tensor.matmul(
                            acc[:],
                            o_sb[:, h_ * L + qt * 128 : h_ * L + (qt + 1) * 128],
                            wo_sb[:, h_ * DOUT + m * QB : h_ * DOUT + (m + 1) * QB],
                            start=(h_ == 0),
                            stop=(h_ == HL - 1),
                        )
                    stage = ap.tile([128, QB], BF16, tag="wstage", bufs=4, name="ws")
                    if on_act:
                        nc.scalar.activation(stage[:], acc[:], COPY)
                    else:
                        nc.vector.tensor_copy(out=stage[:], in_=acc[:])
                    nc.sync.dma_start(
                        out=out[qt * 128 : (qt + 1) * 128, m * QB : (m + 1) * QB],
                        in_=stage[:],
                    )

                for u, (n, h) in enumerate(units):
                    hq_sl = hq_sb[:, h * L + n * QB : h * L + (n + 1) * QB]
                    st = {
                        "nh": (n, h),
                        "ps_o": psp.tile([128, QB], F32, tag="po", name="ps_o"),
                        "ps_d": psp.tile([128, QB], F32, tag="pd", bufs=1, name="ps_d"),
                        "exp_half": [None, None],
                    }
                    state[u] = st
                    wo_emitted = 0
                    tl = {}  # denominator tree: level -> pending tiles
                    for p in range(8):
                        half = p // 4
                        if p % 4 == 0:
                            st["exp_half"][half] = ap.tile(
                                [128, 8 * QB], BF16, tag="exp", bufs=3, name="exp"
                            )
                        off = (p % 4) * 2 * QB
                        ps_s = psp.tile([128, 2 * QB], F32, tag="ps", name="ps_s")
                        for t in range(2):
                            kt = 2 * p + t
                            nc.tensor.matmul(
                                ps_s[:, t * QB : (t + 1) * QB],
                                hk_sb[:, h * L + kt * 128 : h * L + (kt + 1) * 128],
                                hq_sl,
                                start=True,
                                stop=True,
                            )
                        nc.scalar.activation(
                            st["exp_half"][half][:, off : off + 2 * QB], ps_s[:], EXP
                        )
                        # level-0 tree add over this exp pair, fold-up when ready
                        t0 = ap.tile([128, QB], BF16, tag="t0", bufs=2, name="t0")
                        nc.vector.tensor_add(
                            out=t0[:],
                            in0=st["exp_half"][half][:, off : off + QB],
                            in1=st["exp_half"][half][:, off + QB : off + 2 * QB],
                        )
                        tl.setdefault(0, []).append(t0)
                        lv = 0
                        while len(tl.get(lv, [])) == 2:
                            a, b = tl.pop(lv)
                            nxt = ap.tile(
                                [128, QB], BF16, tag=f"t{lv+1}", bufs=2,
                                name=f"t{lv+1}",
                            )
                            nc.vector.tensor_add(out=nxt[:], in0=a[:], in1=b[:])
                            tl.setdefault(lv + 1, []).append(nxt)
                            lv += 1
                        # deferred work from the previous unit / this unit.
                        # AV lags scores by TWO kt-pairs so it never races the
                        # exp pipeline; the last two pairs land in unit u+1.
                        if p == 0 and u > 0:
                            emit_av(u - 1, 6)
                        if p == 1 and u > 0:
                            emit_av(u - 1, 7)
                        if p >= 2:
                            emit_av(u, p - 2)
                        if p == 2 and u > 0:
                            pst = state[u - 1]
                            nc.tensor.matmul(
                                pst["ps_d"][:], ones_sb[:], pst["d128"][:],
                                start=True, stop=True,
                            )
                            n_, h_ = pst["nh"]
                            prev_flush = (h_, n_, pst["ps_o"], pst["ps_d"])
                        if p == 4 and prev_flush is not None:
                            fh, fn, fo, fd = prev_flush
                            flush(prev_flush)
                            prev_flush = None
                            if fh == HL - 1:
                                # block fn fully normalized: queue its Wo groups
                                wo_queue.extend(
                                    (fn * 4 + qq, m)
                                    for qq in range(4)
                                    for m in range(4)
                                )
                        if u < 4:
                            emit_q3(2)
                        elif p >= 2 and p != 4 and wo_queue and wo_emitted < 6:
                            qt_, m_ = wo_queue.pop(0)
                            # drains alternate DVE/ACT to keep both under PE
                            emit_wo_group(qt_, m_, "wo", on_act=(wo_emitted % 2 == 1))
                            wo_emitted += 1
                    st["d128"] = tl[3][0]
                    if u > 0:
                        state.pop(u - 1)
                # drain the pipeline.  Block 3's Wo tail is the only PE work
                # left, and its h=3 inputs depend on the final flush chain —
                # so the first five tail groups' h=0..2 matmuls are emitted
                # FIRST (their inputs are already flushed) to keep the PE busy
                # while ACT finishes the last exps and DVE runs the flushes.
                last = len(units) - 1
                st = state[last]
                wo_queue.extend((3 * 4 + qq, m) for qq in range(4) for m in range(4))
                tail_tags = ["wo", "po", "pd"]

                def wo_mm(acc, qt_, m_, h_):
                    nc.tensor.matmul(
                        acc,
                        o_sb[:, h_ * L + qt_ * 128 : h_ * L + (qt_ + 1) * 128],
                        wo_sb[:, h_ * DOUT + m_ * QB : h_ * DOUT + (m_ + 1) * QB],
                        start=(h_ == 0),
                        stop=(h_ == HL - 1),
                    )

                # NOTE: tag "pd" is NOT usable here — its single bank still
                # belongs to ps_d(last), which the ones-matmul below writes;
                # using it would deadlock PE program order against the flush.
                head = wo_queue[:4]
                ps_pair = psp.tile([128, 2 * QB], F32, tag="ps", name="ps_tail")
                head_accs = [
                    psp.tile([128, QB], F32, tag="wo", bufs=1, name="woacc")[:],
                    psp.tile([128, QB], F32, tag="po", name="woacc")[:],
                    ps_pair[:, 0:QB],
                    ps_pair[:, QB : 2 * QB],
                ]
                for i, (qt_, m_) in enumerate(head):
                    for h_ in range(HL - 1):
                        wo_mm(head_accs[i], qt_, m_, h_)
                # last unit's deferred AV pairs, reduction, final flushes
                emit_av(last, 6)
                emit_av(last, 7)
                nc.tensor.matmul(
                    st["ps_d"][:], ones_sb[:], st["d128"][:], start=True, stop=True
                )
                if prev_flush is not None:  # unit last-1, ones'd at p2 of last
                    flush(prev_flush)
                n_, h_ = st["nh"]
                flush((h_, n_, st["ps_o"], st["ps_d"]))
                for i, (qt_, m_) in enumerate(head):
                    wo_mm(head_accs[i], qt_, m_, HL - 1)
                    stage = ap.tile([128, QB], BF16, tag="wstage", bufs=4, name="ws")
                    nc.scalar.activation(stage[:], head_accs[i], COPY)
                    nc.sync.dma_start(
                        out=out[qt_ * 128 : (qt_ + 1) * 128, m_ * QB : (m_ + 1) * QB],
                        in_=stage[:],
                    )
                for i, (qt_, m_) in enumerate(wo_queue[4:]):
                    emit_wo_group(qt_, m_, tail_tags[i % 3], on_act=(i % 2 == 0))
    nc.compile()
    return nc


def _get_nc():
    if "nc" not in _CACHE:
        _CACHE["nc"] = _build_nc()
    return _CACHE["nc"]


def make_in_maps(query, key_value, Wq, Wk, Wv, Wo):
    scale = 1.0 / math.sqrt(D)
    bf = ml_dtypes.bfloat16
    allones = np.ones((128, 128), bf)
    in_maps = []
    qT = [np.ascontiguousarray(query[b].T.astype(bf)) for b in range(B)]
    kvT = [np.ascontiguousarray(key_value[b].T.astype(bf)) for b in range(B)]
    for core in range(NC_):
        b, g = divmod(core, NC_ // B)
        sl = slice(g * HD, (g + 1) * HD)
        in_maps.append(
            {
                "qT": qT[b],
                "kvT": kvT[b],
                "wqT": np.ascontiguousarray((Wq[sl, :] * scale).T.astype(bf)),
                "wkT": np.ascontiguousarray(Wk[sl, :].T.astype(bf)),
                "wvT": np.ascontiguousarray(Wv[sl, :].T.astype(bf)),
                "woT": np.ascontiguousarray(Wo[:, sl].T.astype(bf)),
                "allones": allones,
            }
        )
    return in_maps


def _numpy_fallback(query, key_value, attention_mask, Wq, Wk, Wv, Wo):
    # Only reached if the mask is not all-ones (never per the problem spec).
    q64, kv64 = query.astype(np.float64), key_value.astype(np.float64)
    hq = (q64 @ Wq.T.astype(np.float64)).reshape(B, L, NH, D).transpose(0, 2, 1, 3)
    hk = (kv64 @ Wk.T.astype(np.float64)).reshape(B, L, NH, D).transpose(0, 2, 1, 3)
    hv = (kv64 @ Wv.T.astype(np.float64)).reshape(B, L, NH, D).transpose(0, 2, 1, 3)
    s = np.einsum("bhqd,bhkd->bhqk", hq, hk) / math.sqrt(D)
    mask = attention_mask[:, None, :, :]
    s = np.where(mask, s, -np.inf)
    s = s - s.max(axis=-1, keepdims=True)
    e = np.exp(s)
    p = e / np.maximum(e.sum(axis=-1, keepdims=True), 1e-300)
    p = np.where(mask, p, 0.0)
    o = np.einsum("bhqk,bhkd->bhqd", p, hv)
    o = o.transpose(0, 2, 1, 3).reshape(B, L, NH * D)
    return (o @ Wo.T.astype(np.float64)).astype(np.float32)


def kernel(query, key_value, attention_mask, Wq, Wk, Wv, Wo):
    query = np.asarray(query)
    key_value = np.asarray(key_value)
    attention_mask = np.asarray(attention_mask)
    Wq, Wk, Wv, Wo = (np.asarray(a) for a in (Wq, Wk, Wv, Wo))

    if not attention_mask.all():
        return _numpy_fallback(query, key_value, attention_mask, Wq, Wk, Wv, Wo)

    from concourse.bass_utils import run_bass_kernel_spmd

    nc = _get_nc()
    in_maps = make_in_maps(query, key_value, Wq, Wk, Wv, Wo)
    res = run_bass_kernel_spmd(nc, in_maps, list(range(NC_))).results
    out = np.zeros((B, L, DOUT), np.float32)
    for core in range(NC_):
        b = core // (NC_ // B)
        out[b] += res[core]["out"].astype(np.float32)
    return out
